# revision 1
# baseline (speedup 1.0000x reference)
"""HRR adapted attention kernel for 8 trn2 cores.

Math (verified vs reference in f64):
  q,k,v = h @ W{q,k,v}.T + b      (per-row, D=2048)
  Qf = rfft(q); Kf = rfft(k)/(|rfft(k)|+eps); Vf likewise
  Mf = causal-cumsum_S(Kf*Vf);  Of = conj(Qf)*Mf;  adapter = irfft(Of)
  out = base + gate*adapter

All FFTs become matmuls: the DFT folds into the projections,
G = W.T @ [C|S] in [d,f] orientation, so the Q/K/V spectra come straight
out of hT.T @ G in a freq-on-partition layout where the causal cumsum is
a native tensor_tensor_scan along the free (sequence) dim.

Sharding: rows (B*S=8192) split 1024/core; the fold is tensor-parallel
over d (256 cols/core) + 6 AllGathers; cross-core cumsum offsets via a
tiny grouped AllGather of per-core block sums.
"""

import numpy as np

import concourse.bass as bass
import concourse.mybir as mybir
import concourse.tile as tile
from concourse import bacc, bass_utils
from concourse.masks import make_identity

F32 = mybir.dt.float32
F32R = mybir.dt.float32r
AX = mybir.AxisListType
ALU = mybir.AluOpType
ACTF = mybir.ActivationFunctionType

B, S, D = 2, 4096, 2048
R = B * S                  # 8192 flat rows
N_CORES = 8
SC = R // N_CORES          # 1024 rows per core
DC = D // N_CORES          # 256 d-columns per core (fold shard)
F = D // 2 + 1             # 1025 rfft bins
# Packed spectrum: col 0 re-plane = DC, col 0 im-plane = Nyquist (both are
# real bins), cols 1..1023 = bins 1..1023 -> exactly 8 tiles of 128.
FP = 1024
NFT = FP // 128            # 8 freq tiles
ND = D // 128              # 16 d tiles
NE = D // 128              # 16 e tiles
NST = SC // 128            # 8 row tiles per core
EPS = 1e-8
FOLD_CHUNKS = [(0, 512), (512, 512)]   # all >=256 for fp32r rate
# mat order; (name, use_sin(ci), w_idx, bias_col)
MATS = [("kre", 0, 1, 2), ("kim", 1, 1, 3),
        ("vre", 0, 2, 4), ("vim", 1, 2, 5),
        ("qre", 0, 0, 0), ("qim", 1, 0, 1)]

_CACHE = {}


def _r(ap):
    return ap.bitcast(F32R)


def _build():
    nc = bacc.Bacc("TRN2", target_bir_lowering=False, debug=False,
                   enable_asserts=False, num_devices=N_CORES)

    h_in = nc.dram_tensor("h", [SC, D], F32, kind="ExternalInput").ap()
    base_in = nc.dram_tensor("base", [SC, D], F32, kind="ExternalInput").ap()
    w_ins = [nc.dram_tensor(f"w{x}", [D, DC], F32, kind="ExternalInput").ap()
             for x in "qkv"]
    cp_in = nc.dram_tensor("cp", [D, FP], F32, kind="ExternalInput").ap()
    sp_in = nc.dram_tensor("sp", [D, FP], F32, kind="ExternalInput").ap()
    am_in = nc.dram_tensor("am", [FP, D], F32, kind="ExternalInput").ap()
    bm_in = nc.dram_tensor("bm", [FP, D], F32, kind="ExternalInput").ap()
    bf_in = nc.dram_tensor("bf", [FP, 6], F32, kind="ExternalInput").ap()
    mask_in = nc.dram_tensor("maskm", [128, 4], F32, kind="ExternalInput").ap()
    gate_in = nc.dram_tensor("gatec", [128, 1], F32, kind="ExternalInput").ap()
    out_t = nc.dram_tensor("out", [SC, D], F32, kind="ExternalOutput").ap()

    with tile.TileContext(nc) as tc, \
         tc.tile_pool(name="pc", bufs=1) as PC, \
         tc.tile_pool(name="psum", bufs=1, space="PSUM") as PP, \
         tc.tile_pool(name="dram", bufs=1, space="DRAM") as DR:

        # ---------- constants ----------
        ident = PC.tile([128, 128], F32, tag="ident")
        make_identity(nc, ident[:])
        mask_sb = PC.tile([128, 4], F32, tag="mask")
        nc.sync.dma_start(mask_sb[:], mask_in[:])
        gate_sb = PC.tile([128, 1], F32, tag="gate")
        nc.sync.dma_start(gate_sb[:], gate_in[:])
        zeros_sb = PC.tile([128, SC], F32, tag="zeros")
        nc.vector.memset(zeros_sb[:], 0.0)
        eps_sb = PC.tile([128, 1], F32, tag="eps")
        nc.vector.memset(eps_sb[:], EPS * EPS)
        bf_sb = PC.tile([128, NFT * 6], F32, tag="bf")
        nc.sync.dma_start(bf_sb[:].rearrange("p (t c) -> p t c", c=6),
                          bf_in.rearrange("(t p) c -> p t c", p=128))

        # ---------- DRAM intermediates ----------
        gin = [DR.tile([DC, FP], F32, tag=f"gin{m}", name=f"gin{m}") for m in range(6)]
        gout = [DR.tile([D, FP], F32, tag=f"gout{m}", name=f"gout{m}", addr_space="Shared") for m in range(6)]
        tin = DR.tile([2 * NFT, 128], F32, tag="tin")
        tout = DR.tile([4 * 2 * NFT, 128], F32, tag="tout")
        q_dram = [DR.tile([FP, SC], F32, tag=f"qd{p}", name=f"qd{p}") for p in range(2)]
        m_dram = [DR.tile([FP, SC], F32, tag=f"md{p}", name=f"md{p}") for p in range(2)]

        with tc.tile_pool(name="pht", bufs=1) as PH:
            hT = [PH.tile([128, SC], F32, tag=f"hT{d}", name=f"hT{d}") for d in range(ND)]

            # ============ fold + h transpose ============
            with tc.tile_pool(name="pf1", bufs=1) as PF1, \
                 tc.tile_pool(name="pf2", bufs=2) as PF2:
                for (f0, fw) in FOLD_CHUNKS:
                    cs_t = {}
                    for ci, cs_in in enumerate((cp_in, sp_in)):
                        for e in range(NE):
                            t = PF1.tile([128, 512], F32, tag=f"cs{ci}_{e}")
                            nc.sync.dma_start(
                                _r(t[:, :fw]),
                                _r(cs_in[e * 128:(e + 1) * 128, f0:f0 + fw]))
                            cs_t[(ci, e)] = t
                    for wi in range(3):
                        w_t = []
                        for e in range(NE):
                            t = PF1.tile([128, DC], F32, tag=f"wt{e}")
                            nc.sync.dma_start(
                                _r(t[:]), _r(w_ins[wi][e * 128:(e + 1) * 128, :]))
                            w_t.append(t)
                        for mi, (_, ci, wj, _) in enumerate(MATS):
                            if wj != wi:
                                continue
                            for dt in range(DC // 128):
                                psf = PP.tile([128, fw], F32, tag="foldp")
                                for e in range(NE):
                                    nc.tensor.matmul(
                                        psf[:],
                                        _r(w_t[e][:, dt * 128:(dt + 1) * 128]),
                                        _r(cs_t[(ci, e)][:, :fw]),
                                        start=(e == 0), stop=(e == NE - 1))
                                gtmp = PF2.tile([128, 512], F32, tag="gtmp")
                                nc.scalar.copy(gtmp[:, :fw], psf[:])
                                nc.sync.dma_start(
                                    gin[mi][dt * 128:(dt + 1) * 128, f0:f0 + fw],
                                    gtmp[:, :fw])

                # h transposes fill the PE while the AllGathers run
                for st in range(NST):
                    hstage = PF2.tile([128, D], F32, tag="hstage")
                    nc.sync.dma_start(hstage[:], h_in[st * 128:(st + 1) * 128, :])
                    for dt in range(ND):
                        pst = PP.tile([128, 128], F32, tag="trp")
                        nc.tensor.transpose(
                            pst[:], hstage[:, dt * 128:(dt + 1) * 128], ident[:])
                        nc.scalar.copy(_r(hT[dt][:, st * 128:(st + 1) * 128]),
                                       _r(pst[:]))

            for mi in range(6):
                nc.gpsimd.collective_compute(
                    "AllGather", ALU.bypass,
                    replica_groups=[list(range(N_CORES))],
                    ins=[gin[mi].opt()], outs=[gout[mi].opt()])

            # ============ projections + bind + scan ============
            with tc.tile_pool(name="pm1", bufs=1) as PM1, \
                 tc.tile_pool(name="pm2", bufs=2) as PM2:

                tots = PM1.tile([128, 2 * NFT], F32, tag="tots")
                planes = {}

                def proj(mi, fts):
                    bcol = MATS[mi][3]
                    psums = {ft: PP.tile([128, SC], F32, tag=f"pp{ft % 3}",
                                          name=f"pp{ft % 3}")
                             for ft in fts}
                    for dt in range(ND):
                        g3 = PM2.tile([128, 128 * len(fts)], F32, tag="g3")
                        nc.sync.dma_start(
                            _r(g3[:]), _r(gout[mi][dt * 128:(dt + 1) * 128,
                                          fts[0] * 128:(fts[-1] + 1) * 128]))
                        for j, ft in enumerate(fts):
                            for nh in range(2):
                                nc.tensor.matmul(
                                    psums[ft][:, nh * 512:(nh + 1) * 512],
                                    _r(g3[:, j * 128:(j + 1) * 128]),
                                    _r(hT[dt][:, nh * 512:(nh + 1) * 512]),
                                    start=(dt == 0), stop=(dt == ND - 1))
                    for ft in fts:
                        pl = PM1.tile([128, SC], F32, tag=f"pl{mi % 4}_{ft % 3}")
                        nc.scalar.activation(
                            pl[:], psums[ft][:], ACTF.Identity,
                            bias=bf_sb[:, ft * 6 + bcol:ft * 6 + bcol + 1])
                        planes[(mi, ft)] = pl

                def norm_bind_scan(ft):
                    kre, kim = planes[(0, ft)], planes[(1, ft)]
                    vre, vim = planes[(2, ft)], planes[(3, ft)]
                    rk = PM2.tile([128, SC], F32, tag="rk")
                    rv = PM2.tile([128, SC], F32, tag="rv")
                    t1 = PM2.tile([128, SC], F32, tag="t1")
                    t2 = PM2.tile([128, SC], F32, tag="t2")
                    for (re, im, rr) in ((kre, kim, rk), (vre, vim, rv)):
                        nc.scalar.square(t1[:], re[:])
                        nc.scalar.square(t2[:], im[:])
                        nc.vector.tensor_add(t1[:], t1[:], t2[:])
                        nc.scalar.activation(rr[:], t1[:], ACTF.Sqrt,
                                             bias=eps_sb[:])
                        nc.vector.reciprocal(rr[:], rr[:])
                    cre = PM2.tile([128, SC], F32, tag="kvre")
                    cim = PM2.tile([128, SC], F32, tag="kvim")
                    nc.vector.tensor_mul(t1[:], kre[:], vre[:])
                    nc.vector.tensor_mul(t2[:], kim[:], vim[:])
                    nc.vector.tensor_sub(cre[:], t1[:], t2[:])
                    nc.vector.tensor_mul(t1[:], kre[:], vim[:])
                    nc.vector.tensor_mul(t2[:], kim[:], vre[:])
                    nc.vector.tensor_add(cim[:], t1[:], t2[:])
                    nc.vector.tensor_mul(rk[:], rk[:], rv[:])
                    nc.vector.tensor_mul(cre[:], cre[:], rk[:])
                    nc.vector.tensor_mul(cim[:], cim[:], rk[:])
                    if ft == 0:
                        # partition row 0 holds two REAL bins (DC in re,
                        # Nyquist in im) -> normalize/bind each separately
                        r0 = []
                        for pl in (kre, kim, vre, vim):
                            rr0 = PM2.tile([1, SC], F32, tag="rr0", bufs=4)
                            nc.scalar.square(rr0[:], pl[0:1, :])
                            nc.scalar.activation(rr0[:], rr0[:], ACTF.Sqrt,
                                                 bias=eps_sb[0:1, :])
                            nc.vector.reciprocal(rr0[:], rr0[:])
                            r0.append(rr0)
                        nc.vector.tensor_mul(cre[0:1, :], kre[0:1, :], vre[0:1, :])
                        nc.vector.tensor_mul(cre[0:1, :], cre[0:1, :], r0[0][:])
                        nc.vector.tensor_mul(cre[0:1, :], cre[0:1, :], r0[2][:])
                        nc.vector.tensor_mul(cim[0:1, :], kim[0:1, :], vim[0:1, :])
                        nc.vector.tensor_mul(cim[0:1, :], cim[0:1, :], r0[1][:])
                        nc.vector.tensor_mul(cim[0:1, :], cim[0:1, :], r0[3][:])
                    # zero-init causal scan (cross-core offset added later);
                    # last column is this core's block total
                    for pi, cv in enumerate((cre, cim)):
                        mt = PM2.tile([128, SC], F32, tag=f"mt{pi}")
                        nc.vector.tensor_tensor_scan(
                            mt[:], cv[:], zeros_sb[:], 0.0, ALU.add, ALU.add)
                        c = 2 * ft + pi
                        nc.vector.tensor_copy(tots[:, c:c + 1], mt[:, SC - 1:SC])
                        nc.sync.dma_start(
                            m_dram[pi][ft * 128:(ft + 1) * 128, :], mt[:])

                FGRPS = [[0, 1, 2], [3, 4, 5], [6, 7]]
                for fts in FGRPS:
                    for mi in range(4):
                        proj(mi, fts)
                    for ft in fts:
                        norm_bind_scan(ft)
                    for mi in (4, 5):          # Q projections -> DRAM spill
                        proj(mi, fts)
                        for ft in fts:
                            nc.sync.dma_start(
                                q_dram[mi - 4][ft * 128:(ft + 1) * 128, :],
                                planes[(mi, ft)][:])

                nc.sync.dma_start(tin.rearrange("c p -> p c"), tots[:])
                nc.gpsimd.collective_compute(
                    "AllGather", ALU.bypass,
                    replica_groups=[[0, 1, 2, 3], [4, 5, 6, 7]],
                    ins=[tin.opt()], outs=[tout.opt()])

        # ============ Of = conj(Qf)*(M + offset), irfft, epilogue ============
        with tc.tile_pool(name="pl1", bufs=1) as PL1, \
             tc.tile_pool(name="pl2", bufs=2) as PL2:
            of = {}
            tout_v = tout.rearrange("(r c) p -> c p r", c=2 * NFT)
            for ft in range(NFT):
                qs, ms, off = [], [], []
                for pi in range(2):
                    q_t = PL2.tile([128, SC], F32, tag=f"qs{pi}")
                    m_t = PL2.tile([128, SC], F32, tag=f"ms{pi}")
                    nc.sync.dma_start(q_t[:], q_dram[pi][ft * 128:(ft + 1) * 128, :])
                    nc.sync.dma_start(m_t[:], m_dram[pi][ft * 128:(ft + 1) * 128, :])
                    c = 2 * ft + pi
                    g4 = PL2.tile([128, 4], F32, tag="g4")
                    nc.sync.dma_start(g4[:], tout_v[c])
                    o_t = PL2.tile([128, 1], F32, tag=f"off{pi}")
                    nc.vector.tensor_mul(g4[:], g4[:], mask_sb[:])
                    nc.vector.tensor_reduce(o_t[:], g4[:], AX.X, ALU.add)
                    nc.vector.tensor_scalar_add(m_t[:], m_t[:], o_t[:])
                    qs.append(q_t); ms.append(m_t)
                t1 = PL2.tile([128, SC], F32, tag="t1")
                t2 = PL2.tile([128, SC], F32, tag="t2")
                ore = PL1.tile([128, SC], F32, tag=f"ore{ft}")
                oim = PL1.tile([128, SC], F32, tag=f"oim{ft}")
                nc.vector.tensor_mul(t1[:], qs[0][:], ms[0][:])
                nc.vector.tensor_mul(t2[:], qs[1][:], ms[1][:])
                nc.vector.tensor_add(_r(ore[:]), t1[:], t2[:])
                nc.vector.tensor_mul(t1[:], qs[0][:], ms[1][:])
                nc.vector.tensor_mul(t2[:], qs[1][:], ms[0][:])
                nc.vector.tensor_sub(_r(oim[:]), t1[:], t2[:])
                if ft == 0:
                    nc.vector.tensor_mul(_r(ore[0:1, :]), qs[0][0:1, :],
                                         ms[0][0:1, :])
                    nc.vector.tensor_mul(_r(oim[0:1, :]), qs[1][0:1, :],
                                         ms[1][0:1, :])
                of[ft] = (ore, oim)

            for dt in range(ND):
                psi = PP.tile([128, SC], F32, tag="pp0")
                at = PL2.tile([128, NFT * 128], F32, tag="at")
                bt = PL2.tile([128, NFT * 128], F32, tag="bt")
                nc.sync.dma_start(
                    _r(at[:].rearrange("p (t d) -> p t d", t=NFT)),
                    _r(am_in[:, dt * 128:(dt + 1) * 128]
                       .rearrange("(t p) d -> p t d", p=128)))
                nc.sync.dma_start(
                    _r(bt[:].rearrange("p (t d) -> p t d", t=NFT)),
                    _r(bm_in[:, dt * 128:(dt + 1) * 128]
                       .rearrange("(t p) d -> p t d", p=128)))
                for ft in range(NFT):
                    ore, oim = of[ft]
                    for nh in range(2):
                        nc.tensor.matmul(
                            psi[:, nh * 512:(nh + 1) * 512],
                            _r(at[:, ft * 128:(ft + 1) * 128]),
                            _r(ore[:, nh * 512:(nh + 1) * 512]),
                            start=(ft == 0), stop=False)
                        nc.tensor.matmul(
                            psi[:, nh * 512:(nh + 1) * 512],
                            _r(bt[:, ft * 128:(ft + 1) * 128]),
                            _r(oim[:, nh * 512:(nh + 1) * 512]),
                            start=False, stop=(ft == NFT - 1))
                adt = PL2.tile([128, SC], F32, tag="adT")
                nc.scalar.copy(adt[:], psi[:])
                for st in range(NST):
                    pst = PP.tile([128, 128], F32, tag="trp")
                    nc.tensor.transpose(
                        pst[:], adt[:, st * 128:(st + 1) * 128], ident[:])
                    btile = PL2.tile([128, 128], F32, tag="btile")
                    otile = PL2.tile([128, 128], F32, tag="otile")
                    nc.sync.dma_start(
                        btile[:], base_in[st * 128:(st + 1) * 128,
                                          dt * 128:(dt + 1) * 128])
                    nc.vector.scalar_tensor_tensor(
                        otile[:], pst[:], gate_sb[:], btile[:],
                        ALU.mult, ALU.add)
                    nc.sync.dma_start(
                        out_t[st * 128:(st + 1) * 128,
                              dt * 128:(dt + 1) * 128], otile[:])

    nc.compile()
    return nc


def _constants():
    e = np.arange(D, dtype=np.float64)
    f = np.arange(FP, dtype=np.float64)
    ang = 2.0 * np.pi * np.outer(e, f) / D           # [e, f]
    cp = np.cos(ang)
    sp = -np.sin(ang)
    sp[:, 0] = np.cos(np.pi * e)                     # Nyquist packed in im col 0
    w = np.full(FP, 2.0)
    w[0] = 1.0
    angA = 2.0 * np.pi * np.outer(f, e) / D          # [f, d]
    am = (w[:, None] / D) * np.cos(angA)
    bm = -(w[:, None] / D) * np.sin(angA)
    bm[0, :] = np.cos(np.pi * e) / D                 # Nyquist inverse row
    return (cp.astype(np.float32), sp.astype(np.float32),
            am.astype(np.float32), bm.astype(np.float32))


def _run(inputs, trace=False):
    if "nc" not in _CACHE:
        _CACHE["nc"] = _build()
    nc = _CACHE["nc"]
    cp, sp, am, bm = _CACHE.setdefault("const", _constants())

    h = np.ascontiguousarray(np.asarray(inputs["hidden_states"],
                                        np.float32).reshape(R, D))
    base = np.ascontiguousarray(np.asarray(inputs["base_output"],
                                           np.float32).reshape(R, D))
    gate = np.asarray(inputs["gate"], np.float32).reshape(-1)[0]

    bf = np.zeros((FP, 6), np.float32)
    for j, bn in enumerate(("bq", "bk", "bv")):
        spec = np.fft.rfft(np.asarray(inputs[bn], np.float64))
        bf[:FP, 2 * j] = spec.real[:FP].astype(np.float32)
        bf[:FP, 2 * j + 1] = spec.imag[:FP].astype(np.float32)
        bf[0, 2 * j + 1] = np.float32(spec.real[F - 1])
    gate_col = np.full((128, 1), gate, np.float32)

    ws = {x: np.asarray(inputs[f"W{x}"], np.float32) for x in "qkv"}
    in_maps = []
    for c in range(N_CORES):
        mask = np.zeros((128, 4), np.float32)
        mask[:, :c % 4] = 1.0
        in_maps.append({
            "h": h[c * SC:(c + 1) * SC],
            "base": base[c * SC:(c + 1) * SC],
            "wq": np.ascontiguousarray(ws["q"][:, c * DC:(c + 1) * DC]),
            "wk": np.ascontiguousarray(ws["k"][:, c * DC:(c + 1) * DC]),
            "wv": np.ascontiguousarray(ws["v"][:, c * DC:(c + 1) * DC]),
            "cp": cp, "sp": sp, "am": am, "bm": bm, "bf": bf,
            "maskm": mask, "gatec": gate_col,
        })

    res = bass_utils.run_bass_kernel_spmd(
        nc, in_maps, core_ids=list(range(N_CORES)), trace=trace)
    out = np.concatenate([res.results[c]["out"] for c in range(N_CORES)], axis=0)
    return out.reshape(B, S, D).astype(np.float32), res


def kernel(**inputs) -> np.ndarray:
    out, _ = _run(inputs, trace=False)
    return out



# revision 2
# speedup vs baseline: 2.9377x; 2.9377x over previous
"""HRR adapted attention kernel for 8 trn2 cores.

Math (verified vs reference in f64):
  q,k,v = h @ W{q,k,v}.T + b      (per-row, D=2048)
  Qf = rfft(q); Kf = rfft(k)/(|rfft(k)|+eps); Vf likewise
  Mf = causal-cumsum_S(Kf*Vf);  Of = conj(Qf)*Mf;  adapter = irfft(Of)
  out = base + gate*adapter

All FFTs become matmuls: the DFT folds into the projections,
G = W.T @ [C|S] in [d,f] orientation, so the Q/K/V spectra come straight
out of hT.T @ G in a freq-on-partition layout where the causal cumsum is
a native tensor_tensor_scan along the free (sequence) dim.

Sharding: 2 batch groups x 4 spectrum shards. Core c handles batch c//4
and 256 packed rfft bins (2 f-tiles of 128). Each core folds only its own
spectrum slice (full W needed, no fold collective), projects / binds /
scans all 4096 rows of its batch locally, and computes a PARTIAL inverse
DFT over its f-slice. One grouped bf16 ReduceScatter (d-sharded, split in
two halves for overlap) combines the partials; the epilogue adds base in
the transposed [d, s] layout so no on-chip transposes are needed at all.

The packed spectrum keeps rfft bins DC and Nyquist in the re/im planes of
packed column 0 (both real). Their special normalize/bind/unbind algebra
is expressed uniformly via per-partition {0,1} mask columns, so the SPMD
program is identical on every core.
"""

import numpy as np

import concourse.bass as bass
import concourse.mybir as mybir
import concourse.tile as tile
from concourse import bacc, bass_utils

F32 = mybir.dt.float32
BF16 = mybir.dt.bfloat16
AX = mybir.AxisListType
ALU = mybir.AluOpType
ACTF = mybir.ActivationFunctionType

B, S, D = 2, 4096, 2048
N_CORES = 8
NG, GS = 2, 4              # batch groups x spectrum shards
FP = 1024                  # packed rfft bins (col0: re=DC, im=Nyquist)
FBLK = FP // GS            # 256 packed bins per core
NFT = FBLK // 128          # 2 local f-tiles
ND = D // 128              # 16 d tiles
NE = D // 128              # 16 e tiles
DQ = D // GS               # 512 output d rows per core
SCH = 512                  # sequence chunk for the pipeline
NSC = S // SCH             # 8 chunks
EPS = 1e-8
# mat order: (name, use_sin(ci), w_idx, bias_col)
MATS = [("kre", 0, 1, 2), ("kim", 1, 1, 3),
        ("vre", 0, 2, 4), ("vim", 1, 2, 5),
        ("qre", 0, 0, 0), ("qim", 1, 0, 1)]

_CACHE = {}


def _build():
    nc = bacc.Bacc("TRN2", target_bir_lowering=False, debug=False,
                   enable_asserts=False, num_devices=N_CORES)

    hT_in = nc.dram_tensor("ht", [D, S], BF16, kind="ExternalInput").ap()
    # W pre-tiled host-side: [dt, e, c] so each d-column block is contiguous
    w_ins = [nc.dram_tensor(f"w{x}", [ND * D, 128], BF16,
                            kind="ExternalInput").ap() for x in "qkv"]
    csl_in = nc.dram_tensor("csl", [D, 2 * FBLK], BF16, kind="ExternalInput").ap()
    am_in = nc.dram_tensor("am2", [FBLK, D], BF16, kind="ExternalInput").ap()
    bm_in = nc.dram_tensor("bm2", [FBLK, D], BF16, kind="ExternalInput").ap()
    bfc_in = nc.dram_tensor("bfc", [128, NFT * 6], F32, kind="ExternalInput").ap()
    # cols: mz_ft0, mn_ft0, mz_ft1, mn_ft1, gate
    mz_in = nc.dram_tensor("mzg", [128, 5], F32, kind="ExternalInput").ap()
    baseT_in = nc.dram_tensor("baseT", [DQ, S], F32, kind="ExternalInput").ap()
    outT = nc.dram_tensor("outT", [DQ, S], F32, kind="ExternalOutput").ap()

    with nc.allow_low_precision("bf16 spectra; scan state stays fp32"), \
         tile.TileContext(nc) as tc, \
         tc.tile_pool(name="pc", bufs=1) as PC, \
         tc.tile_pool(name="dram", bufs=1, space="DRAM") as DR:

        # ---------- constants ----------
        mz_sb = PC.tile([128, 5], F32, tag="mz")
        nc.sync.dma_start(mz_sb[:], mz_in[:])
        bfc_sb = PC.tile([128, NFT * 6], F32, tag="bfc")
        nc.sync.dma_start(bfc_sb[:], bfc_in[:])
        eps_sb = PC.tile([128, 1], F32, tag="eps")
        nc.vector.memset(eps_sb[:], EPS * EPS)
        zeros_bf = PC.tile([128, SCH], BF16, tag="zer")
        nc.vector.memset(zeros_bf[:], 0.0)

        # ---------- DRAM intermediates ----------
        part_a = DR.tile([D // 2, S], BF16, tag="pa", name="pa")
        part_b = DR.tile([D // 2, S], BF16, tag="pb", name="pb")
        rs_a = DR.tile([DQ // 2, S], BF16, tag="ra", name="ra")
        rs_b = DR.tile([DQ // 2, S], BF16, tag="rb", name="rb")

        # persistent unbound-spectra planes (filled chunk-wise)
        with tc.tile_pool(name="pgl", bufs=1) as PGL:
            ofp = {}
            for ftl in range(NFT):
                for pi in range(2):
                    ofp[(ftl, pi)] = PGL.tile([128, S], BF16,
                                              tag=f"of{ftl}{pi}",
                                              name=f"of{ftl}{pi}")

            # ============ fold: G[d,f-slice] = W.T @ [C|S] ============
            with tc.tile_pool(name="pgG", bufs=1) as PGG:
                G = [PGG.tile([128, ND * 512], BF16, tag=f"G{wi}",
                              name=f"G{wi}") for wi in range(3)]
                with tc.tile_pool(name="pf", bufs=2) as PF, \
                     tc.tile_pool(name="psf", bufs=1, space="PSUM") as PPF:
                    csl_sb = PF.tile([128, NE * 512], BF16, tag="csl", bufs=1)
                    nc.sync.dma_start(
                        csl_sb[:].rearrange("p (t c) -> p t c", c=512),
                        csl_in.rearrange("(t p) c -> p t c", p=128))
                    for wi in range(3):
                        for dt in range(ND):
                            w_sb = PF.tile([128, NE * 128], BF16, tag="wsl")
                            nc.sync.dma_start(
                                w_sb[:].rearrange("p (t c) -> p t c", c=128),
                                w_ins[wi][dt * D:(dt + 1) * D, :]
                                .rearrange("(t p) c -> p t c", p=128))
                            psf = PPF.tile([128, 512], F32, tag=f"pf{dt % 2}")
                            for e in range(NE):
                                nc.tensor.matmul(
                                    psf[:],
                                    w_sb[:, e * 128:(e + 1) * 128],
                                    csl_sb[:, e * 512:(e + 1) * 512],
                                    start=(e == 0), stop=(e == NE - 1))
                            nc.scalar.copy(
                                G[wi][:, dt * 512:(dt + 1) * 512], psf[:])

                # ============ s-chunk pipeline ============
                with tc.tile_pool(name="pm", bufs=2) as PM, \
                     tc.tile_pool(name="psm", bufs=1, space="PSUM") as PPM:
                    m_prev = {}
                    for sc in range(NSC):
                        s0, s1 = sc * SCH, (sc + 1) * SCH
                        hts = []
                        for dt in range(ND):
                            ht = PM.tile([128, SCH], BF16, tag=f"ht{dt}")
                            nc.sync.dma_start(
                                ht[:], hT_in[dt * 128:(dt + 1) * 128, s0:s1])
                            hts.append(ht)
                        planes = {}
                        for mi, (_, ci, wi, bcol) in enumerate(MATS):
                            for ftl in range(NFT):
                                ps = PPM.tile([128, SCH], F32,
                                              tag=f"pp{(2 * mi + ftl) % 2}")
                                off = ci * 256 + ftl * 128
                                for dt in range(ND):
                                    nc.tensor.matmul(
                                        ps[:],
                                        G[wi][:, dt * 512 + off:
                                              dt * 512 + off + 128],
                                        hts[dt][:],
                                        start=(dt == 0), stop=(dt == ND - 1))
                                pl = PM.tile([128, SCH], BF16,
                                             tag=f"pl{mi}_{ftl}")
                                nc.scalar.activation(
                                    pl[:], ps[:], ACTF.Identity,
                                    bias=bfc_sb[:, ftl * 6 + bcol:
                                                ftl * 6 + bcol + 1])
                                planes[(mi, ftl)] = pl

                        for ftl in range(NFT):
                            mz = mz_sb[:, 2 * ftl:2 * ftl + 1]
                            mn = mz_sb[:, 2 * ftl + 1:2 * ftl + 2]
                            kre, kim = planes[(0, ftl)], planes[(1, ftl)]
                            vre, vim = planes[(2, ftl)], planes[(3, ftl)]
                            qre, qim = planes[(4, ftl)], planes[(5, ftl)]
                            # --- unit-magnitude norms (masked for the two
                            # real bins packed in partition 0 of ft 0) ---
                            rr = {}
                            for pj, (re_, im_) in enumerate(((kre, kim),
                                                            (vre, vim))):
                                sq0 = PM.tile([128, SCH], BF16, tag="sq0")
                                sq1 = PM.tile([128, SCH], BF16, tag="sq1")
                                nc.scalar.square(sq0[:], re_[:])
                                nc.scalar.square(sq1[:], im_[:])
                                ra = PM.tile([128, SCH], BF16, tag=f"ra{pj}")
                                rb = PM.tile([128, SCH], BF16, tag=f"rb{pj}")
                                nc.vector.scalar_tensor_tensor(
                                    ra[:], sq1[:], mz, sq0[:],
                                    ALU.mult, ALU.add)
                                nc.vector.scalar_tensor_tensor(
                                    rb[:], sq0[:], mz, sq1[:],
                                    ALU.mult, ALU.add)
                                nc.scalar.activation(ra[:], ra[:], ACTF.Sqrt,
                                                     bias=eps_sb[:])
                                nc.scalar.activation(rb[:], rb[:], ACTF.Sqrt,
                                                     bias=eps_sb[:])
                                nc.vector.reciprocal(ra[:], ra[:])
                                nc.vector.reciprocal(rb[:], rb[:])
                                rr[pj] = (ra, rb)
                            # --- bind: cre+i*cim = Kn * Vn (masked) ---
                            u0 = PM.tile([128, SCH], BF16, tag="u0")
                            u1 = PM.tile([128, SCH], BF16, tag="u1")
                            t0 = PM.tile([128, SCH], BF16, tag="t0")
                            t1 = PM.tile([128, SCH], BF16, tag="t1")
                            cre = PM.tile([128, SCH], BF16, tag="cre")
                            cim = PM.tile([128, SCH], BF16, tag="cim")
                            nc.vector.tensor_mul(u0[:], kre[:], vre[:])
                            nc.vector.tensor_mul(u1[:], kim[:], vim[:])
                            nc.vector.scalar_tensor_tensor(
                                cre[:], u1[:], mn, u0[:], ALU.mult, ALU.add)
                            nc.vector.tensor_mul(t0[:], kre[:], vim[:])
                            nc.vector.tensor_mul(t1[:], kim[:], vre[:])
                            nc.vector.tensor_add(t0[:], t0[:], t1[:])
                            nc.vector.tensor_sub(t0[:], t0[:], u1[:])
                            nc.vector.scalar_tensor_tensor(
                                cim[:], t0[:], mz, u1[:], ALU.mult, ALU.add)
                            kra, krb = rr[0]
                            vra, vrb = rr[1]
                            nc.vector.tensor_mul(kra[:], kra[:], vra[:])
                            nc.vector.tensor_mul(krb[:], krb[:], vrb[:])
                            nc.vector.tensor_mul(cre[:], cre[:], kra[:])
                            nc.vector.tensor_mul(cim[:], cim[:], krb[:])
                            # --- causal scan (fp32 state + fp32 carry) ---
                            ms = []
                            for pi, cv in enumerate((cre, cim)):
                                mt = PM.tile([128, SCH], F32, tag=f"m{ftl}{pi}")
                                init = (0.0 if sc == 0
                                        else m_prev[(ftl, pi)][:, SCH - 1:SCH])
                                nc.vector.tensor_tensor_scan(
                                    mt[:], cv[:], zeros_bf[:], init,
                                    ALU.add, ALU.add)
                                m_prev[(ftl, pi)] = mt
                                ms.append(mt)
                            # --- unbind: Of = conj(Qf) * Mf (masked) ---
                            nc.vector.tensor_mul(u0[:], qre[:], ms[0][:])
                            nc.vector.tensor_mul(u1[:], qim[:], ms[1][:])
                            nc.vector.scalar_tensor_tensor(
                                ofp[(ftl, 0)][:, s0:s1], u1[:], mz, u0[:],
                                ALU.mult, ALU.add)
                            nc.vector.tensor_mul(t0[:], qre[:], ms[1][:])
                            nc.vector.tensor_mul(t1[:], qim[:], ms[0][:])
                            nc.vector.tensor_sub(t0[:], t0[:], t1[:])
                            nc.vector.tensor_sub(t0[:], t0[:], u1[:])
                            nc.vector.scalar_tensor_tensor(
                                ofp[(ftl, 1)][:, s0:s1], t0[:], mz, u1[:],
                                ALU.mult, ALU.add)

            # ============ partial inverse DFT + ReduceScatter ============
            with tc.tile_pool(name="pi", bufs=2) as PI, \
                 tc.tile_pool(name="psi", bufs=1, space="PSUM") as PPI:
                ab = {}
                for ftl in range(NFT):
                    amt = PI.tile([128, D], BF16, tag=f"am{ftl}", bufs=1)
                    nc.sync.dma_start(
                        amt[:], am_in[ftl * 128:(ftl + 1) * 128, :])
                    bmt = PI.tile([128, D], BF16, tag=f"bm{ftl}", bufs=1)
                    nc.sync.dma_start(
                        bmt[:], bm_in[ftl * 128:(ftl + 1) * 128, :])
                    ab[ftl] = (amt, bmt)
                for dhalf in range(2):
                    part = (part_a, part_b)[dhalf]
                    for dt8 in range(ND // 2):
                        dt = dhalf * 8 + dt8
                        for sh in range(2):
                            psi = PPI.tile([128, S // 2], F32, tag=f"pi{sh}")
                            for cc in range(S // 2 // 512):
                                scol = sh * (S // 2) + cc * 512
                                step = 0
                                for ftl in range(NFT):
                                    amt, bmt = ab[ftl]
                                    for pi, abt in ((0, amt), (1, bmt)):
                                        nc.tensor.matmul(
                                            psi[:, cc * 512:(cc + 1) * 512],
                                            abt[:, dt * 128:(dt + 1) * 128],
                                            ofp[(ftl, pi)][:, scol:scol + 512],
                                            start=(step == 0),
                                            stop=(step == 2 * NFT - 1))
                                        step += 1
                            stg = PI.tile([128, S // 2], BF16, tag=f"st{sh}")
                            nc.scalar.copy(stg[:], psi[:])
                            nc.sync.dma_start(
                                part[dt8 * 128:(dt8 + 1) * 128,
                                     sh * (S // 2):(sh + 1) * (S // 2)],
                                stg[:])
                    rs = (rs_a, rs_b)[dhalf]
                    nc.gpsimd.collective_compute(
                        "ReduceScatter", ALU.add,
                        replica_groups=[[0, 1, 2, 3], [4, 5, 6, 7]],
                        ins=[part.opt()], outs=[rs.opt()])

        # ============ epilogue: out = base + gate * adapter ============
        with tc.tile_pool(name="pe", bufs=2) as PE2:
            for t in range(DQ // 128):
                src = (rs_a, rs_b)[t // 2]
                row = (t % 2) * 128
                ad = PE2.tile([128, S], BF16, tag="ad")
                nc.sync.dma_start(ad[:], src[row:row + 128, :])
                bs = PE2.tile([128, S], F32, tag="bs")
                nc.sync.dma_start(bs[:], baseT_in[t * 128:(t + 1) * 128, :])
                ot = PE2.tile([128, S], F32, tag="ot")
                nc.vector.scalar_tensor_tensor(
                    ot[:], ad[:], mz_sb[:, 4:5], bs[:], ALU.mult, ALU.add)
                nc.sync.dma_start(outT[t * 128:(t + 1) * 128, :], ot[:])

    nc.compile()
    return nc


def _constants():
    npbf = mybir.dt.np(BF16)
    e = np.arange(D, dtype=np.float64)
    f = np.arange(FP, dtype=np.float64)
    ang = 2.0 * np.pi * np.outer(e, f) / D           # [e, f]
    cp = np.cos(ang)
    sp = -np.sin(ang)
    sp[:, 0] = np.cos(np.pi * e)                     # Nyquist packed in im col 0
    w = np.full(FP, 2.0)
    w[0] = 1.0
    angA = 2.0 * np.pi * np.outer(f, e) / D          # [f, d]
    am = (w[:, None] / D) * np.cos(angA)
    bm = -(w[:, None] / D) * np.sin(angA)
    bm[0, :] = np.cos(np.pi * e) / D                 # Nyquist inverse row
    return (cp.astype(npbf), sp.astype(npbf),
            am.astype(npbf), bm.astype(npbf))


def _run(inputs, trace=False):
    if "nc" not in _CACHE:
        _CACHE["nc"] = _build()
    nc = _CACHE["nc"]
    npbf = mybir.dt.np(BF16)
    cp, sp, am, bm = _CACHE.setdefault("const", _constants())

    h = np.asarray(inputs["hidden_states"], np.float32).reshape(B, S, D)
    base = np.asarray(inputs["base_output"], np.float32).reshape(B, S, D)
    gate = float(np.asarray(inputs["gate"], np.float32).reshape(-1)[0])

    bf = np.zeros((FP, 6), np.float32)
    for j, bn in enumerate(("bq", "bk", "bv")):
        spec = np.fft.rfft(np.asarray(inputs[bn], np.float64))
        bf[:FP, 2 * j] = spec.real[:FP].astype(np.float32)
        bf[:FP, 2 * j + 1] = spec.imag[:FP].astype(np.float32)
        bf[0, 2 * j + 1] = np.float32(spec.real[FP])

    # W pre-tiled: [dt, e, c]
    wt = {}
    for x in "qkv":
        wf = np.asarray(inputs[f"W{x}"], np.float32)
        wt[x] = np.ascontiguousarray(
            wf.reshape(D, ND, 128).transpose(1, 0, 2)).astype(npbf) \
            .reshape(ND * D, 128)

    hT = [np.ascontiguousarray(h[g].T).astype(npbf) for g in range(NG)]

    in_maps = []
    for c in range(N_CORES):
        g, r = c // GS, c % GS
        blk = slice(r * FBLK, (r + 1) * FBLK)
        csl = np.concatenate([cp[:, blk], sp[:, blk]], axis=1)
        bfc = np.empty((128, NFT * 6), np.float32)
        for ftl in range(NFT):
            bfc[:, ftl * 6:(ftl + 1) * 6] = \
                bf[r * FBLK + ftl * 128: r * FBLK + (ftl + 1) * 128]
        mzg = np.ones((128, 5), np.float32)
        if r == 0:
            mzg[0, 0] = 0.0          # ft0 partition 0: DC/Nyquist real bins
        mzg[:, 1] = -mzg[:, 0]
        mzg[:, 3] = -mzg[:, 2]
        mzg[:, 4] = gate
        baseT = np.ascontiguousarray(np.concatenate(
            [base[g][:, 256 * r:256 * (r + 1)].T,
             base[g][:, 1024 + 256 * r:1024 + 256 * (r + 1)].T], axis=0))
        in_maps.append({
            "ht": hT[g],
            "wq": wt["q"], "wk": wt["k"], "wv": wt["v"],
            "csl": np.ascontiguousarray(csl),
            "am2": np.ascontiguousarray(am[blk]),
            "bm2": np.ascontiguousarray(bm[blk]),
            "bfc": bfc, "mzg": mzg, "baseT": baseT,
        })

    res = bass_utils.run_bass_kernel_spmd(
        nc, in_maps, core_ids=list(range(N_CORES)), trace=trace)

    out = np.empty((B, S, D), np.float32)
    for c in range(N_CORES):
        g, r = c // GS, c % GS
        o = res.results[c]["outT"]
        out[g][:, 256 * r:256 * (r + 1)] = o[0:256].T
        out[g][:, 1024 + 256 * r:1024 + 256 * (r + 1)] = o[256:512].T
    return out, res


def kernel(**inputs) -> np.ndarray:
    out, _ = _run(inputs)
    return out


# revision 13
# speedup vs baseline: 2.9748x; 1.0126x over previous
"""HRR adapted attention kernel for 8 trn2 cores.

Math (verified vs reference in f64):
  q,k,v = h @ W{q,k,v}.T + b      (per-row, D=2048)
  Qf = rfft(q); Kf = rfft(k)/(|rfft(k)|+eps); Vf likewise
  Mf = causal-cumsum_S(Kf*Vf);  Of = conj(Qf)*Mf;  adapter = irfft(Of)
  out = base + gate*adapter

All FFTs become matmuls: the DFT folds into the projections,
G = W.T @ [C|S] in [d,f] orientation, so the Q/K/V spectra come straight
out of hT.T @ G in a freq-on-partition layout where the causal cumsum is
a native tensor_tensor_scan along the free (sequence) dim.

Sharding: 2 batch groups x 4 spectrum shards. Core c handles batch c//4
and 256 packed rfft bins (2 f-tiles of 128). Each core folds only its own
spectrum slice (full W needed, no fold collective), projects / binds /
scans all 4096 rows of its batch locally, and computes a PARTIAL inverse
DFT over its f-slice. One grouped bf16 ReduceScatter (d-sharded, split in
two halves for overlap) combines the partials; the epilogue adds base in
the transposed [d, s] layout so no on-chip transposes are needed at all.

The packed spectrum keeps rfft bins DC and Nyquist in the re/im planes of
packed column 0 (both real). Their special normalize/bind/unbind algebra
is expressed uniformly via per-partition {0,1} mask columns, so the SPMD
program is identical on every core.
"""

import numpy as np

import concourse.bass as bass
import concourse.mybir as mybir
import concourse.tile as tile
from concourse import bacc, bass_utils

F32 = mybir.dt.float32
BF16 = mybir.dt.bfloat16
AX = mybir.AxisListType
ALU = mybir.AluOpType
ACTF = mybir.ActivationFunctionType

B, S, D = 2, 4096, 2048
N_CORES = 8
NG, GS = 2, 4              # batch groups x spectrum shards
FP = 1024                  # packed rfft bins (col0: re=DC, im=Nyquist)
FBLK = FP // GS            # 256 packed bins per core
NFT = FBLK // 128          # 2 local f-tiles
ND = D // 128              # 16 d tiles
NE = D // 128              # 16 e tiles
DQ = D // GS               # 512 output d rows per core
SCH = 512                  # sequence chunk for the pipeline
NSC = S // SCH             # 8 chunks
EPS = 1e-8
# mat order: (name, use_sin(ci), w_idx, bias_col)
MATS = [("kre", 0, 1, 2), ("kim", 1, 1, 3),
        ("vre", 0, 2, 4), ("vim", 1, 2, 5),
        ("qre", 0, 0, 0), ("qim", 1, 0, 1)]

_CACHE = {}


def _build():
    nc = bacc.Bacc("TRN2", target_bir_lowering=False, debug=False,
                   enable_asserts=False, num_devices=N_CORES)

    hT_in = nc.dram_tensor("ht", [D, S], BF16, kind="ExternalInput").ap()
    # W pre-tiled host-side into the exact SBUF image per d-block:
    # w[dt, p, t*128+c] = W[t*128+p, dt*128+c] -> contiguous 4KB rows
    w_ins = [nc.dram_tensor(f"w{x}", [ND * 128, NE * 128], BF16,
                            kind="ExternalInput").ap() for x in "qkv"]
    csl_in = nc.dram_tensor("csl", [D, 2 * FBLK], BF16, kind="ExternalInput").ap()
    am_in = nc.dram_tensor("am2", [FBLK, D], BF16, kind="ExternalInput").ap()
    bm_in = nc.dram_tensor("bm2", [FBLK, D], BF16, kind="ExternalInput").ap()
    bfc_in = nc.dram_tensor("bfc", [128, NFT * 6], F32, kind="ExternalInput").ap()
    # cols: mz_ft0, mn_ft0, mz_ft1, mn_ft1, gate
    mz_in = nc.dram_tensor("mzg", [128, 5], F32, kind="ExternalInput").ap()
    baseT_in = nc.dram_tensor("baseT", [DQ, S], F32, kind="ExternalInput").ap()
    outT = nc.dram_tensor("outT", [DQ, S], F32, kind="ExternalOutput").ap()

    with nc.allow_low_precision("bf16 spectra; scan state stays fp32"), \
         tile.TileContext(nc) as tc, \
         tc.tile_pool(name="pc", bufs=1) as PC, \
         tc.tile_pool(name="dram", bufs=1, space="DRAM") as DR:

        # ---------- constants ----------
        mz_sb = PC.tile([128, 5], F32, tag="mz")
        nc.sync.dma_start(mz_sb[:], mz_in[:])
        bfc_sb = PC.tile([128, NFT * 6], F32, tag="bfc")
        nc.sync.dma_start(bfc_sb[:], bfc_in[:])
        eps_sb = PC.tile([128, 1], F32, tag="eps")
        nc.vector.memset(eps_sb[:], EPS * EPS)
        zeros_bf = PC.tile([128, SCH], BF16, tag="zer")
        nc.vector.memset(zeros_bf[:], 0.0)

        # ---------- DRAM intermediates ----------
        part_a = DR.tile([D // 2, S], BF16, tag="pa", name="pa")
        part_b = DR.tile([D // 2, S], BF16, tag="pb", name="pb")
        rs_a = DR.tile([DQ // 2, S], BF16, tag="ra", name="ra")
        rs_b = DR.tile([DQ // 2, S], BF16, tag="rb", name="rb")

        # persistent unbound-spectra planes (filled chunk-wise) + iDFT mats
        with tc.tile_pool(name="pgl", bufs=1) as PGL:
            ofp = {}
            for ftl in range(NFT):
                for pi in range(2):
                    ofp[(ftl, pi)] = PGL.tile([128, S], BF16,
                                              tag=f"of{ftl}{pi}",
                                              name=f"of{ftl}{pi}")
            ab = {}
            for ftl in range(NFT):
                amt = PGL.tile([128, D], BF16, tag=f"am{ftl}", name=f"am{ftl}")
                nc.sync.dma_start(amt[:], am_in[ftl * 128:(ftl + 1) * 128, :])
                bmt = PGL.tile([128, D], BF16, tag=f"bm{ftl}", name=f"bm{ftl}")
                nc.sync.dma_start(bmt[:], bm_in[ftl * 128:(ftl + 1) * 128, :])
                ab[ftl] = (amt, bmt)

            # ============ fold: G[d,f-slice] = W.T @ [C|S] ============
            with tc.tile_pool(name="pgG", bufs=1) as PGG:
                G = [PGG.tile([128, ND * 512], BF16, tag=f"G{wi}",
                              name=f"G{wi}") for wi in range(3)]
                with tc.tile_pool(name="pf", bufs=3) as PF, \
                     tc.tile_pool(name="psf", bufs=1, space="PSUM") as PPF:
                    csl_sb = PF.tile([128, NE * 512], BF16, tag="csl", bufs=1)
                    nc.sync.dma_start(
                        csl_sb[:].rearrange("p (t c) -> p t c", c=512),
                        csl_in.rearrange("(t p) c -> p t c", p=128))
                    for wi in range(3):
                        for dt in range(ND):
                            w_sb = PF.tile([128, NE * 128], BF16, tag="wsl")
                            nc.sync.dma_start(
                                w_sb[:],
                                w_ins[wi][dt * 128:(dt + 1) * 128, :])
                            psf = PPF.tile([128, 512], F32, tag=f"pf{dt % 2}")
                            for e in range(NE):
                                nc.tensor.matmul(
                                    psf[:],
                                    w_sb[:, e * 128:(e + 1) * 128],
                                    csl_sb[:, e * 512:(e + 1) * 512],
                                    start=(e == 0), stop=(e == NE - 1))
                            nc.scalar.copy(
                                G[wi][:, dt * 512:(dt + 1) * 512], psf[:])

                # ============ s-chunk pipeline ============
                with tc.tile_pool(name="pm", bufs=2) as PM, \
                     tc.tile_pool(name="psm", bufs=1, space="PSUM") as PPM:
                    m_prev = {}
                    ofv = {}

                    def load_htc(sc):
                        htc = PM.tile([128, NE * SCH], BF16, tag="htc")
                        nc.sync.dma_start(
                            htc[:].rearrange("p (t s) -> p t s", s=SCH),
                            hT_in[:, sc * SCH:(sc + 1) * SCH]
                            .rearrange("(t p) s -> p t s", p=128))
                        return htc

                    def irfft_a(sc):
                        # partial inverse DFT, d rows 0:1024, this s-chunk
                        s0 = sc * SCH
                        for dt in range(ND // 2):
                            psi = PPM.tile([128, SCH], F32,
                                           tag=f"ir{dt % 2}")
                            step = 0
                            for ftl in range(NFT):
                                amt, bmt = ab[ftl]
                                for pi, abt in ((0, amt), (1, bmt)):
                                    nc.tensor.matmul(
                                        psi[:],
                                        abt[:, dt * 128:(dt + 1) * 128],
                                        ofv[(sc, ftl, pi)],
                                        start=(step == 0),
                                        stop=(step == 2 * NFT - 1))
                                    step += 1
                            stg = PM.tile([128, SCH], BF16, tag=f"sta{dt % 2}")
                            nc.scalar.copy(stg[:], psi[:])
                            nc.scalar.dma_start(
                                part_a[dt * 128:(dt + 1) * 128,
                                       s0:s0 + SCH], stg[:])

                    htc = load_htc(0)
                    for sc in range(NSC):
                        s0, s1 = sc * SCH, (sc + 1) * SCH
                        htc_next = load_htc(sc + 1) if sc + 1 < NSC else None
                        planes = {}
                        for mi, (_, ci, wi, bcol) in enumerate(MATS):
                            for ftl in range(NFT):
                                ps = PPM.tile([128, SCH], F32,
                                              tag=f"pp{(2 * mi + ftl) % 2}")
                                off = ci * 256 + ftl * 128
                                for dt in range(ND):
                                    nc.tensor.matmul(
                                        ps[:],
                                        G[wi][:, dt * 512 + off:
                                              dt * 512 + off + 128],
                                        htc[:, dt * SCH:(dt + 1) * SCH],
                                        start=(dt == 0), stop=(dt == ND - 1))
                                pl = PM.tile([128, SCH], BF16,
                                             tag=f"pl{mi}_{ftl}")
                                nc.scalar.activation(
                                    pl[:], ps[:], ACTF.Identity,
                                    bias=bfc_sb[:, ftl * 6 + bcol:
                                                ftl * 6 + bcol + 1])
                                planes[(mi, ftl)] = pl
                        # previous chunk's partial iDFT keeps PE busy while
                        # the DVE works through this chunk's planes
                        if sc > 0:
                            irfft_a(sc - 1)

                        for ftl in range(NFT):
                            mz = mz_sb[:, 2 * ftl:2 * ftl + 1]
                            mn = mz_sb[:, 2 * ftl + 1:2 * ftl + 2]
                            kre, kim = planes[(0, ftl)], planes[(1, ftl)]
                            vre, vim = planes[(2, ftl)], planes[(3, ftl)]
                            qre, qim = planes[(4, ftl)], planes[(5, ftl)]
                            # --- unit-magnitude norms (masked for the two
                            # real bins packed in partition 0 of ft 0) ---
                            rr = {}
                            for pj, (re_, im_) in enumerate(((kre, kim),
                                                            (vre, vim))):
                                sq0 = PM.tile([128, SCH], BF16, tag="sq0")
                                sq1 = PM.tile([128, SCH], BF16, tag="sq1")
                                nc.scalar.square(sq0[:], re_[:])
                                nc.scalar.square(sq1[:], im_[:])
                                ra = PM.tile([128, SCH], BF16, tag=f"ra{pj}")
                                rb = PM.tile([128, SCH], BF16, tag=f"rb{pj}")
                                nc.vector.scalar_tensor_tensor(
                                    ra[:], sq1[:], mz, sq0[:],
                                    ALU.mult, ALU.add)
                                nc.vector.scalar_tensor_tensor(
                                    rb[:], sq0[:], mz, sq1[:],
                                    ALU.mult, ALU.add)
                                nc.scalar.activation(ra[:], ra[:], ACTF.Sqrt,
                                                     bias=eps_sb[:])
                                nc.scalar.activation(rb[:], rb[:], ACTF.Sqrt,
                                                     bias=eps_sb[:])
                                nc.vector.reciprocal(ra[:], ra[:])
                                nc.vector.reciprocal(rb[:], rb[:])
                                rr[pj] = (ra, rb)
                            # --- bind: cre+i*cim = Kn * Vn (masked) ---
                            u0 = PM.tile([128, SCH], BF16, tag="u0")
                            u1 = PM.tile([128, SCH], BF16, tag="u1")
                            t0 = PM.tile([128, SCH], BF16, tag="t0")
                            t1 = PM.tile([128, SCH], BF16, tag="t1")
                            cre = PM.tile([128, SCH], BF16, tag="cre")
                            cim = PM.tile([128, SCH], BF16, tag="cim")
                            nc.vector.tensor_mul(u0[:], kre[:], vre[:])
                            nc.vector.tensor_mul(u1[:], kim[:], vim[:])
                            nc.vector.scalar_tensor_tensor(
                                cre[:], u1[:], mn, u0[:], ALU.mult, ALU.add)
                            nc.vector.tensor_mul(t0[:], kre[:], vim[:])
                            nc.vector.tensor_mul(t1[:], kim[:], vre[:])
                            nc.vector.tensor_add(t0[:], t0[:], t1[:])
                            nc.vector.tensor_sub(t0[:], t0[:], u1[:])
                            nc.vector.scalar_tensor_tensor(
                                cim[:], t0[:], mz, u1[:], ALU.mult, ALU.add)
                            kra, krb = rr[0]
                            vra, vrb = rr[1]
                            nc.vector.tensor_mul(kra[:], kra[:], vra[:])
                            nc.vector.tensor_mul(krb[:], krb[:], vrb[:])
                            nc.vector.tensor_mul(cre[:], cre[:], kra[:])
                            nc.vector.tensor_mul(cim[:], cim[:], krb[:])
                            # --- causal scan (fp32 state, bf16 carry) ---
                            ms = []
                            for pi, cv in enumerate((cre, cim)):
                                mt = PM.tile([128, SCH], BF16,
                                             tag=f"m{ftl}{pi}")
                                init = (0.0 if sc == 0
                                        else m_prev[(ftl, pi)][:, SCH - 1:SCH])
                                nc.vector.tensor_tensor_scan(
                                    mt[:], cv[:], zeros_bf[:], init,
                                    ALU.add, ALU.add)
                                m_prev[(ftl, pi)] = mt
                                ms.append(mt)
                            # --- unbind: Of = conj(Qf) * Mf (masked) ---
                            ofv[(sc, ftl, 0)] = ofp[(ftl, 0)][:, s0:s1]
                            ofv[(sc, ftl, 1)] = ofp[(ftl, 1)][:, s0:s1]
                            nc.vector.tensor_mul(u0[:], qre[:], ms[0][:])
                            nc.vector.tensor_mul(u1[:], qim[:], ms[1][:])
                            nc.vector.scalar_tensor_tensor(
                                ofv[(sc, ftl, 0)], u1[:], mz, u0[:],
                                ALU.mult, ALU.add)
                            nc.vector.tensor_mul(t0[:], qre[:], ms[1][:])
                            nc.vector.tensor_mul(t1[:], qim[:], ms[0][:])
                            nc.vector.tensor_sub(t0[:], t0[:], t1[:])
                            nc.vector.tensor_sub(t0[:], t0[:], u1[:])
                            nc.vector.scalar_tensor_tensor(
                                ofv[(sc, ftl, 1)], t0[:], mz, u1[:],
                                ALU.mult, ALU.add)
                        htc = htc_next
                    irfft_a(NSC - 1)

            # ============ RS#1 + second-half inverse DFT + RS#2 ============
            nc.gpsimd.collective_compute(
                "ReduceScatter", ALU.add,
                replica_groups=[[0, 1, 2, 3], [4, 5, 6, 7]],
                ins=[part_a.opt()], outs=[rs_a.opt()])
            with tc.tile_pool(name="pi", bufs=2) as PI, \
                 tc.tile_pool(name="psi", bufs=1, space="PSUM") as PPI:
                for dt8 in range(ND // 2):
                    dt = 8 + dt8
                    for sh in range(2):
                        psi = PPI.tile([128, S // 2], F32, tag=f"pi{sh}")
                        for cc in range(S // 2 // 512):
                            scol = sh * (S // 2) + cc * 512
                            step = 0
                            for ftl in range(NFT):
                                amt, bmt = ab[ftl]
                                for pi, abt in ((0, amt), (1, bmt)):
                                    nc.tensor.matmul(
                                        psi[:, cc * 512:(cc + 1) * 512],
                                        abt[:, dt * 128:(dt + 1) * 128],
                                        ofp[(ftl, pi)][:, scol:scol + 512],
                                        start=(step == 0),
                                        stop=(step == 2 * NFT - 1))
                                    step += 1
                        stg = PI.tile([128, S // 2], BF16, tag=f"st{sh}")
                        nc.scalar.copy(stg[:], psi[:])
                        nc.scalar.dma_start(
                            part_b[dt8 * 128:(dt8 + 1) * 128,
                                   sh * (S // 2):(sh + 1) * (S // 2)],
                            stg[:])
                nc.gpsimd.collective_compute(
                    "ReduceScatter", ALU.add,
                    replica_groups=[[0, 1, 2, 3], [4, 5, 6, 7]],
                    ins=[part_b.opt()], outs=[rs_b.opt()])

        # ============ epilogue: out = base + gate * adapter ============
        with tc.tile_pool(name="pe", bufs=2) as PE2:
            for t in range(DQ // 128):
                src = (rs_a, rs_b)[t // 2]
                row = (t % 2) * 128
                ad = PE2.tile([128, S], BF16, tag="ad")
                nc.sync.dma_start(ad[:], src[row:row + 128, :])
                bs = PE2.tile([128, S], F32, tag="bs")
                nc.sync.dma_start(bs[:], baseT_in[t * 128:(t + 1) * 128, :])
                ot = PE2.tile([128, S], F32, tag="ot")
                nc.vector.scalar_tensor_tensor(
                    ot[:], ad[:], mz_sb[:, 4:5], bs[:], ALU.mult, ALU.add)
                nc.scalar.dma_start(outT[t * 128:(t + 1) * 128, :], ot[:])

    nc.compile()
    return nc


def _constants():
    npbf = mybir.dt.np(BF16)
    e = np.arange(D, dtype=np.float64)
    f = np.arange(FP, dtype=np.float64)
    ang = 2.0 * np.pi * np.outer(e, f) / D           # [e, f]
    cp = np.cos(ang)
    sp = -np.sin(ang)
    sp[:, 0] = np.cos(np.pi * e)                     # Nyquist packed in im col 0
    w = np.full(FP, 2.0)
    w[0] = 1.0
    angA = 2.0 * np.pi * np.outer(f, e) / D          # [f, d]
    am = (w[:, None] / D) * np.cos(angA)
    bm = -(w[:, None] / D) * np.sin(angA)
    bm[0, :] = np.cos(np.pi * e) / D                 # Nyquist inverse row
    return (cp.astype(npbf), sp.astype(npbf),
            am.astype(npbf), bm.astype(npbf))


def _run(inputs, trace=False):
    if "nc" not in _CACHE:
        _CACHE["nc"] = _build()
    nc = _CACHE["nc"]
    npbf = mybir.dt.np(BF16)
    cp, sp, am, bm = _CACHE.setdefault("const", _constants())

    h = np.asarray(inputs["hidden_states"], np.float32).reshape(B, S, D)
    base = np.asarray(inputs["base_output"], np.float32).reshape(B, S, D)
    gate = float(np.asarray(inputs["gate"], np.float32).reshape(-1)[0])

    bf = np.zeros((FP, 6), np.float32)
    for j, bn in enumerate(("bq", "bk", "bv")):
        spec = np.fft.rfft(np.asarray(inputs[bn], np.float64))
        bf[:FP, 2 * j] = spec.real[:FP].astype(np.float32)
        bf[:FP, 2 * j + 1] = spec.imag[:FP].astype(np.float32)
        bf[0, 2 * j + 1] = np.float32(spec.real[FP])

    # W pre-tiled into the SBUF image: w[dt, p, t*128+c] = W[t*128+p, dt*128+c]
    wt = {}
    for x in "qkv":
        wf = np.asarray(inputs[f"W{x}"], np.float32)
        wt[x] = np.ascontiguousarray(
            wf.reshape(NE, 128, ND, 128).transpose(2, 1, 0, 3)
            .reshape(ND * 128, NE * 128)).astype(npbf)

    hT = [np.ascontiguousarray(h[g].T).astype(npbf) for g in range(NG)]

    in_maps = []
    for c in range(N_CORES):
        g, r = c // GS, c % GS
        blk = slice(r * FBLK, (r + 1) * FBLK)
        csl = np.concatenate([cp[:, blk], sp[:, blk]], axis=1)
        bfc = np.empty((128, NFT * 6), np.float32)
        for ftl in range(NFT):
            bfc[:, ftl * 6:(ftl + 1) * 6] = \
                bf[r * FBLK + ftl * 128: r * FBLK + (ftl + 1) * 128]
        mzg = np.ones((128, 5), np.float32)
        if r == 0:
            mzg[0, 0] = 0.0          # ft0 partition 0: DC/Nyquist real bins
        mzg[:, 1] = -mzg[:, 0]
        mzg[:, 3] = -mzg[:, 2]
        mzg[:, 4] = gate
        baseT = np.ascontiguousarray(np.concatenate(
            [base[g][:, 256 * r:256 * (r + 1)].T,
             base[g][:, 1024 + 256 * r:1024 + 256 * (r + 1)].T], axis=0))
        in_maps.append({
            "ht": hT[g],
            "wq": wt["q"], "wk": wt["k"], "wv": wt["v"],
            "csl": np.ascontiguousarray(csl),
            "am2": np.ascontiguousarray(am[blk]),
            "bm2": np.ascontiguousarray(bm[blk]),
            "bfc": bfc, "mzg": mzg, "baseT": baseT,
        })

    res = bass_utils.run_bass_kernel_spmd(
        nc, in_maps, core_ids=list(range(N_CORES)), trace=trace)

    out = np.empty((B, S, D), np.float32)
    for c in range(N_CORES):
        g, r = c // GS, c % GS
        o = res.results[c]["outT"]
        out[g][:, 256 * r:256 * (r + 1)] = o[0:256].T
        out[g][:, 1024 + 256 * r:1024 + 256 * (r + 1)] = o[256:512].T
    return out, res


def kernel(**inputs) -> np.ndarray:
    out, _ = _run(inputs)
    return out


# revision 26
# speedup vs baseline: 3.1488x; 1.0585x over previous
"""HRR adapted attention kernel for 8 trn2 cores.

Math (verified vs reference in f64):
  q,k,v = h @ W{q,k,v}.T + b      (per-row, D=2048)
  Qf = rfft(q); Kf = rfft(k)/(|rfft(k)|+eps); Vf likewise
  Mf = causal-cumsum_S(Kf*Vf);  Of = conj(Qf)*Mf;  adapter = irfft(Of)
  out = base + gate*adapter

All FFTs become matmuls: the DFT folds into the projections,
G = W.T @ [C|S] in [d,f] orientation, so the Q/K/V spectra come straight
out of hT.T @ G in a freq-on-partition layout where the causal cumsum is
a native tensor_tensor_scan along the free (sequence) dim.

Sharding: 2 batch groups x 4 spectrum shards. Core c handles batch c//4
and 256 packed rfft bins (2 f-tiles of 128). Each core folds only its own
spectrum slice (full W needed, no fold collective), projects / binds /
scans all 4096 rows of its batch locally, and computes a PARTIAL inverse
DFT over its f-slice. One grouped bf16 ReduceScatter (d-sharded, split in
two halves for overlap) combines the partials; the epilogue adds base in
the transposed [d, s] layout so no on-chip transposes are needed at all.

The packed spectrum keeps rfft bins DC and Nyquist in the re/im planes of
packed column 0 (both real). Their special normalize/bind/unbind algebra
is expressed uniformly via per-partition {0,1} mask columns, so the SPMD
program is identical on every core.
"""

import numpy as np

import concourse.bass as bass
import concourse.mybir as mybir
import concourse.tile as tile
from concourse import bacc, bass_utils

F32 = mybir.dt.float32
BF16 = mybir.dt.bfloat16
NSQ = 4                    # ReduceScatter split: one piece per 2 s-chunks
AX = mybir.AxisListType
ALU = mybir.AluOpType
ACTF = mybir.ActivationFunctionType

B, S, D = 2, 4096, 2048
N_CORES = 8
NG, GS = 2, 4              # batch groups x spectrum shards
FP = 1024                  # packed rfft bins (col0: re=DC, im=Nyquist)
FBLK = FP // GS            # 256 packed bins per core
NFT = FBLK // 128          # 2 local f-tiles
ND = D // 128              # 16 d tiles
NE = D // 128              # 16 e tiles
DQ = D // GS               # 512 output d rows per core
SCH = 512                  # sequence chunk for the pipeline
NSC = S // SCH             # 8 chunks
EPS = 1e-8
# mat order: (name, use_sin(ci), w_idx, bias_col)
MATS = [("kre", 0, 1, 2), ("kim", 1, 1, 3),
        ("vre", 0, 2, 4), ("vim", 1, 2, 5),
        ("qre", 0, 0, 0), ("qim", 1, 0, 1)]

_CACHE = {}


def _build():
    nc = bacc.Bacc("TRN2", target_bir_lowering=False, debug=False,
                   enable_asserts=False, num_devices=N_CORES)

    hT_in = nc.dram_tensor("ht", [D, S], BF16, kind="ExternalInput").ap()
    # W pre-tiled host-side into the exact SBUF image per d-block:
    # w[dt, p, t*128+c] = W[t*128+p, dt*128+c] -> contiguous 4KB rows
    w_ins = [nc.dram_tensor(f"w{x}", [ND * 128, NE * 128], BF16,
                            kind="ExternalInput").ap() for x in "qkv"]
    csl_in = nc.dram_tensor("csl", [D, 2 * FBLK], BF16, kind="ExternalInput").ap()
    am_in = nc.dram_tensor("am2", [FBLK, D], BF16, kind="ExternalInput").ap()
    bm_in = nc.dram_tensor("bm2", [FBLK, D], BF16, kind="ExternalInput").ap()
    bfc_in = nc.dram_tensor("bfc", [128, NFT * 6], F32, kind="ExternalInput").ap()
    # cols: mz_ft0, mn_ft0, mz_ft1, mn_ft1, gate
    mz_in = nc.dram_tensor("mzg", [128, 5], F32, kind="ExternalInput").ap()
    baseT_in = nc.dram_tensor("baseT", [DQ, S], F32, kind="ExternalInput").ap()
    outT = nc.dram_tensor("outT", [DQ, S], F32, kind="ExternalOutput").ap()

    with nc.allow_low_precision("bf16 spectra; scan state stays fp32"), \
         tile.TileContext(nc) as tc, \
         tc.tile_pool(name="pc", bufs=1) as PC, \
         tc.tile_pool(name="dram", bufs=1, space="DRAM") as DR:

        # ---------- constants ----------
        mz_sb = PC.tile([128, 5], F32, tag="mz")
        nc.sync.dma_start(mz_sb[:], mz_in[:])
        bfc_sb = PC.tile([128, NFT * 6], F32, tag="bfc")
        nc.sync.dma_start(bfc_sb[:], bfc_in[:])
        eps_sb = PC.tile([128, 1], F32, tag="eps")
        nc.vector.memset(eps_sb[:], EPS * EPS)
        zeros_bf = PC.tile([128, SCH], BF16, tag="zer")
        nc.vector.memset(zeros_bf[:], 0.0)

        # ---------- DRAM intermediates ----------
        SQ = S // NSQ
        parts = [DR.tile([D, SQ], BF16, tag=f"pa{q}", name=f"pa{q}")
                 for q in range(NSQ)]
        rss = [DR.tile([DQ, SQ], BF16, tag=f"rs{q}", name=f"rs{q}")
               for q in range(NSQ)]

        # persistent unbound-spectra planes (filled chunk-wise) + iDFT mats
        with tc.tile_pool(name="pgl", bufs=1) as PGL:
            ofp = {}
            for ftl in range(NFT):
                for pi in range(2):
                    ofp[(ftl, pi)] = PGL.tile([128, S], BF16,
                                              tag=f"of{ftl}{pi}",
                                              name=f"of{ftl}{pi}")
            ab = {}
            for ftl in range(NFT):
                amt = PGL.tile([128, D], BF16, tag=f"am{ftl}", name=f"am{ftl}")
                nc.sync.dma_start(amt[:], am_in[ftl * 128:(ftl + 1) * 128, :])
                bmt = PGL.tile([128, D], BF16, tag=f"bm{ftl}", name=f"bm{ftl}")
                nc.sync.dma_start(bmt[:], bm_in[ftl * 128:(ftl + 1) * 128, :])
                ab[ftl] = (amt, bmt)

            # ============ fold: G[d,f-slice] = W.T @ [C|S] ============
            with tc.tile_pool(name="pgG", bufs=1) as PGG:
                G = [PGG.tile([128, ND * 512], BF16, tag=f"G{wi}",
                              name=f"G{wi}") for wi in range(3)]
                with tc.tile_pool(name="pf", bufs=3) as PF, \
                     tc.tile_pool(name="psf", bufs=1, space="PSUM") as PPF:
                    csl_sb = PF.tile([128, NE * 512], BF16, tag="csl", bufs=1)
                    nc.sync.dma_start(
                        csl_sb[:].rearrange("p (t c) -> p t c", c=512),
                        csl_in.rearrange("(t p) c -> p t c", p=128))
                    for wi in range(3):
                        for dt in range(ND):
                            w_sb = PF.tile([128, NE * 128], BF16, tag="wsl")
                            nc.sync.dma_start(
                                w_sb[:],
                                w_ins[wi][dt * 128:(dt + 1) * 128, :])
                            psf = PPF.tile([128, 512], F32, tag=f"pf{dt % 2}")
                            for e in range(NE):
                                nc.tensor.matmul(
                                    psf[:],
                                    w_sb[:, e * 128:(e + 1) * 128],
                                    csl_sb[:, e * 512:(e + 1) * 512],
                                    start=(e == 0), stop=(e == NE - 1))
                            nc.scalar.copy(
                                G[wi][:, dt * 512:(dt + 1) * 512], psf[:])

                # ============ s-chunk pipeline ============
                with tc.tile_pool(name="pm", bufs=2) as PM, \
                     tc.tile_pool(name="psm", bufs=1, space="PSUM") as PPM:
                    m_prev = {}
                    ofv = {}

                    def load_htc(sc):
                        htc = PM.tile([128, NE * SCH], BF16, tag="htc")
                        nc.sync.dma_start(
                            htc[:].rearrange("p (t s) -> p t s", s=SCH),
                            hT_in[:, sc * SCH:(sc + 1) * SCH]
                            .rearrange("(t p) s -> p t s", p=128))
                        return htc

                    def irfft_sc(sc):
                        # full-depth partial inverse DFT for one s-chunk;
                        # lands in the s-quarter partial tensor sc // 2
                        part = parts[sc // 2]
                        pcol = (sc % 2) * SCH
                        for dt in range(ND):
                            psi = PPM.tile([128, SCH], F32,
                                           tag=f"ir{dt % 2}")
                            step = 0
                            for ftl in range(NFT):
                                amt, bmt = ab[ftl]
                                for pi, abt in ((0, amt), (1, bmt)):
                                    nc.tensor.matmul(
                                        psi[:],
                                        abt[:, dt * 128:(dt + 1) * 128],
                                        ofv[(sc, ftl, pi)],
                                        start=(step == 0),
                                        stop=(step == 2 * NFT - 1))
                                    step += 1
                            stg = PM.tile([128, SCH], BF16, tag=f"sta{dt % 2}")
                            nc.scalar.copy(stg[:], psi[:])
                            nc.scalar.dma_start(
                                part[dt * 128:(dt + 1) * 128,
                                     pcol:pcol + SCH], stg[:])

                    def rs_q(q):
                        nc.gpsimd.collective_compute(
                            "ReduceScatter", ALU.add,
                            replica_groups=[[0, 1, 2, 3], [4, 5, 6, 7]],
                            ins=[parts[q].opt()], outs=[rss[q].opt()])

                    htc = load_htc(0)
                    for sc in range(NSC):
                        s0, s1 = sc * SCH, (sc + 1) * SCH
                        htc_next = load_htc(sc + 1) if sc + 1 < NSC else None
                        # an older chunk's partial iDFT keeps PE busy while
                        # the DVE works through the recent chunks' planes
                        if sc > 1:
                            irfft_sc(sc - 2)
                            if sc % 2 == 1:
                                rs_q(sc // 2 - 1)   # quarter complete
                        planes = {}
                        for mi, (_, ci, wi, bcol) in enumerate(MATS):
                            for ftl in range(NFT):
                                ps = PPM.tile([128, SCH], F32,
                                              tag=f"pp{(2 * mi + ftl) % 2}")
                                off = ci * 256 + ftl * 128
                                for dt in range(ND):
                                    nc.tensor.matmul(
                                        ps[:],
                                        G[wi][:, dt * 512 + off:
                                              dt * 512 + off + 128],
                                        htc[:, dt * SCH:(dt + 1) * SCH],
                                        start=(dt == 0), stop=(dt == ND - 1))
                                pl = PM.tile([128, SCH], BF16,
                                             tag=f"pl{mi}_{ftl}")
                                nc.scalar.activation(
                                    pl[:], ps[:], ACTF.Identity,
                                    bias=bfc_sb[:, ftl * 6 + bcol:
                                                ftl * 6 + bcol + 1])
                                planes[(mi, ftl)] = pl
                        for ftl in range(NFT):
                            mz = mz_sb[:, 2 * ftl:2 * ftl + 1]
                            mn = mz_sb[:, 2 * ftl + 1:2 * ftl + 2]
                            kre, kim = planes[(0, ftl)], planes[(1, ftl)]
                            vre, vim = planes[(2, ftl)], planes[(3, ftl)]
                            qre, qim = planes[(4, ftl)], planes[(5, ftl)]
                            # --- unit-magnitude norms (masked for the two
                            # real bins packed in partition 0 of ft 0) ---
                            rr = {}
                            for pj, (re_, im_) in enumerate(((kre, kim),
                                                            (vre, vim))):
                                sq0 = PM.tile([128, SCH], BF16, tag="sq0")
                                sq1 = PM.tile([128, SCH], BF16, tag="sq1")
                                nc.scalar.square(sq0[:], re_[:])
                                nc.scalar.square(sq1[:], im_[:])
                                ra = PM.tile([128, SCH], BF16, tag=f"ra{pj}")
                                rb = PM.tile([128, SCH], BF16, tag=f"rb{pj}")
                                nc.vector.scalar_tensor_tensor(
                                    ra[:], sq1[:], mz, sq0[:],
                                    ALU.mult, ALU.add)
                                nc.vector.scalar_tensor_tensor(
                                    rb[:], sq0[:], mz, sq1[:],
                                    ALU.mult, ALU.add)
                                nc.scalar.activation(ra[:], ra[:], ACTF.Sqrt,
                                                     bias=eps_sb[:])
                                nc.scalar.activation(rb[:], rb[:], ACTF.Sqrt,
                                                     bias=eps_sb[:])
                                nc.vector.reciprocal(ra[:], ra[:])
                                nc.vector.reciprocal(rb[:], rb[:])
                                rr[pj] = (ra, rb)
                            # --- bind: cre+i*cim = Kn * Vn (masked) ---
                            u0 = PM.tile([128, SCH], BF16, tag="u0")
                            u1 = PM.tile([128, SCH], BF16, tag="u1")
                            t0 = PM.tile([128, SCH], BF16, tag="t0")
                            t1 = PM.tile([128, SCH], BF16, tag="t1")
                            cre = PM.tile([128, SCH], BF16, tag="cre")
                            cim = PM.tile([128, SCH], BF16, tag="cim")
                            nc.vector.tensor_mul(u0[:], kre[:], vre[:])
                            nc.vector.tensor_mul(u1[:], kim[:], vim[:])
                            nc.vector.scalar_tensor_tensor(
                                cre[:], u1[:], mn, u0[:], ALU.mult, ALU.add)
                            nc.vector.tensor_mul(t0[:], kre[:], vim[:])
                            nc.vector.tensor_mul(t1[:], kim[:], vre[:])
                            nc.vector.tensor_add(t0[:], t0[:], t1[:])
                            nc.vector.tensor_sub(t0[:], t0[:], u1[:])
                            nc.vector.scalar_tensor_tensor(
                                cim[:], t0[:], mz, u1[:], ALU.mult, ALU.add)
                            kra, krb = rr[0]
                            vra, vrb = rr[1]
                            nc.vector.tensor_mul(kra[:], kra[:], vra[:])
                            nc.vector.tensor_mul(krb[:], krb[:], vrb[:])
                            nc.vector.tensor_mul(cre[:], cre[:], kra[:])
                            nc.vector.tensor_mul(cim[:], cim[:], krb[:])
                            # --- causal scan (fp32 state, bf16 carry) ---
                            ms = []
                            for pi, cv in enumerate((cre, cim)):
                                mt = PM.tile([128, SCH], BF16,
                                             tag=f"m{ftl}{pi}")
                                init = (0.0 if sc == 0
                                        else m_prev[(ftl, pi)][:, SCH - 1:SCH])
                                nc.vector.tensor_tensor_scan(
                                    mt[:], cv[:], zeros_bf[:], init,
                                    ALU.add, ALU.add)
                                m_prev[(ftl, pi)] = mt
                                ms.append(mt)
                            # --- unbind: Of = conj(Qf) * Mf (masked) ---
                            ofv[(sc, ftl, 0)] = ofp[(ftl, 0)][:, s0:s1]
                            ofv[(sc, ftl, 1)] = ofp[(ftl, 1)][:, s0:s1]
                            nc.vector.tensor_mul(u0[:], qre[:], ms[0][:])
                            nc.vector.tensor_mul(u1[:], qim[:], ms[1][:])
                            nc.vector.scalar_tensor_tensor(
                                ofv[(sc, ftl, 0)], u1[:], mz, u0[:],
                                ALU.mult, ALU.add)
                            nc.vector.tensor_mul(t0[:], qre[:], ms[1][:])
                            nc.vector.tensor_mul(t1[:], qim[:], ms[0][:])
                            nc.vector.tensor_sub(t0[:], t0[:], t1[:])
                            nc.vector.tensor_sub(t0[:], t0[:], u1[:])
                            nc.vector.scalar_tensor_tensor(
                                ofv[(sc, ftl, 1)], t0[:], mz, u1[:],
                                ALU.mult, ALU.add)
                        htc = htc_next
                    irfft_sc(NSC - 2)
                    rs_q(NSQ - 2)
                    irfft_sc(NSC - 1)
                    rs_q(NSQ - 1)

            # ============ epilogue: out = base + gate * adapter ============
            # s-quarter order: rs outputs arrive q0..q3; base prefetched
            with tc.tile_pool(name="pe", bufs=1) as PE2:
                SQ = S // NSQ
                bss = []
                for t in range(DQ // 128):
                    bs = PE2.tile([128, S], F32, tag=f"bs{t}")
                    nc.sync.dma_start(
                        bs[:], baseT_in[t * 128:(t + 1) * 128, :])
                    bss.append(bs)
                for q in range(NSQ):
                    for t in range(DQ // 128):
                        ad = PE2.tile([128, SQ], BF16, tag=f"ad{t % 2}",
                                      bufs=2)
                        nc.sync.dma_start(
                            ad[:], rss[q][t * 128:(t + 1) * 128, :])
                        ot = PE2.tile([128, SQ], F32, tag=f"ot{t % 2}",
                                      bufs=2)
                        nc.vector.scalar_tensor_tensor(
                            ot[:], ad[:], mz_sb[:, 4:5],
                            bss[t][:, q * SQ:(q + 1) * SQ],
                            ALU.mult, ALU.add)
                        nc.scalar.dma_start(
                            outT[t * 128:(t + 1) * 128, q * SQ:(q + 1) * SQ],
                            ot[:])

    nc.compile()
    return nc


def _constants():
    npbf = mybir.dt.np(BF16)
    e = np.arange(D, dtype=np.float64)
    f = np.arange(FP, dtype=np.float64)
    ang = 2.0 * np.pi * np.outer(e, f) / D           # [e, f]
    cp = np.cos(ang)
    sp = -np.sin(ang)
    sp[:, 0] = np.cos(np.pi * e)                     # Nyquist packed in im col 0
    w = np.full(FP, 2.0)
    w[0] = 1.0
    angA = 2.0 * np.pi * np.outer(f, e) / D          # [f, d]
    am = (w[:, None] / D) * np.cos(angA)
    bm = -(w[:, None] / D) * np.sin(angA)
    bm[0, :] = np.cos(np.pi * e) / D                 # Nyquist inverse row
    return (cp.astype(npbf), sp.astype(npbf),
            am.astype(npbf), bm.astype(npbf))


def _run(inputs, trace=False):
    if "nc" not in _CACHE:
        _CACHE["nc"] = _build()
    nc = _CACHE["nc"]
    npbf = mybir.dt.np(BF16)
    cp, sp, am, bm = _CACHE.setdefault("const", _constants())

    h = np.asarray(inputs["hidden_states"], np.float32).reshape(B, S, D)
    base = np.asarray(inputs["base_output"], np.float32).reshape(B, S, D)
    gate = float(np.asarray(inputs["gate"], np.float32).reshape(-1)[0])

    bf = np.zeros((FP, 6), np.float32)
    for j, bn in enumerate(("bq", "bk", "bv")):
        spec = np.fft.rfft(np.asarray(inputs[bn], np.float64))
        bf[:FP, 2 * j] = spec.real[:FP].astype(np.float32)
        bf[:FP, 2 * j + 1] = spec.imag[:FP].astype(np.float32)
        bf[0, 2 * j + 1] = np.float32(spec.real[FP])

    # W pre-tiled into the SBUF image: w[dt, p, t*128+c] = W[t*128+p, dt*128+c]
    wt = {}
    for x in "qkv":
        wf = np.asarray(inputs[f"W{x}"], np.float32)
        wt[x] = np.ascontiguousarray(
            wf.reshape(NE, 128, ND, 128).transpose(2, 1, 0, 3)
            .reshape(ND * 128, NE * 128)).astype(npbf)

    hT = [np.ascontiguousarray(h[g].T).astype(npbf) for g in range(NG)]

    in_maps = []
    for c in range(N_CORES):
        g, r = c // GS, c % GS
        blk = slice(r * FBLK, (r + 1) * FBLK)
        csl = np.concatenate([cp[:, blk], sp[:, blk]], axis=1)
        bfc = np.empty((128, NFT * 6), np.float32)
        for ftl in range(NFT):
            bfc[:, ftl * 6:(ftl + 1) * 6] = \
                bf[r * FBLK + ftl * 128: r * FBLK + (ftl + 1) * 128]
        mzg = np.ones((128, 5), np.float32)
        if r == 0:
            mzg[0, 0] = 0.0          # ft0 partition 0: DC/Nyquist real bins
        mzg[:, 1] = -mzg[:, 0]
        mzg[:, 3] = -mzg[:, 2]
        mzg[:, 4] = gate
        baseT = np.ascontiguousarray(base[g][:, DQ * r:DQ * (r + 1)].T)
        in_maps.append({
            "ht": hT[g],
            "wq": wt["q"], "wk": wt["k"], "wv": wt["v"],
            "csl": np.ascontiguousarray(csl),
            "am2": np.ascontiguousarray(am[blk]),
            "bm2": np.ascontiguousarray(bm[blk]),
            "bfc": bfc, "mzg": mzg, "baseT": baseT,
        })

    res = bass_utils.run_bass_kernel_spmd(
        nc, in_maps, core_ids=list(range(N_CORES)), trace=trace)

    out = np.empty((B, S, D), np.float32)
    for c in range(N_CORES):
        g, r = c // GS, c % GS
        out[g][:, DQ * r:DQ * (r + 1)] = res.results[c]["outT"].T
    return out, res


def kernel(**inputs) -> np.ndarray:
    out, _ = _run(inputs)
    return out


# revision 27
# speedup vs baseline: 3.1625x; 1.0044x over previous
"""HRR adapted attention kernel for 8 trn2 cores.

Math (verified vs reference in f64):
  q,k,v = h @ W{q,k,v}.T + b      (per-row, D=2048)
  Qf = rfft(q); Kf = rfft(k)/(|rfft(k)|+eps); Vf likewise
  Mf = causal-cumsum_S(Kf*Vf);  Of = conj(Qf)*Mf;  adapter = irfft(Of)
  out = base + gate*adapter

All FFTs become matmuls: the DFT folds into the projections,
G = W.T @ [C|S] in [d,f] orientation, so the Q/K/V spectra come straight
out of hT.T @ G in a freq-on-partition layout where the causal cumsum is
a native tensor_tensor_scan along the free (sequence) dim.

Sharding: 2 batch groups x 4 spectrum shards. Core c handles batch c//4
and 256 packed rfft bins (2 f-tiles of 128). Each core folds only its own
spectrum slice (full W needed, no fold collective), projects / binds /
scans all 4096 rows of its batch locally, and computes a PARTIAL inverse
DFT over its f-slice. One grouped bf16 ReduceScatter (d-sharded, split in
two halves for overlap) combines the partials; the epilogue adds base in
the transposed [d, s] layout so no on-chip transposes are needed at all.

The packed spectrum keeps rfft bins DC and Nyquist in the re/im planes of
packed column 0 (both real). Their special normalize/bind/unbind algebra
is expressed uniformly via per-partition {0,1} mask columns, so the SPMD
program is identical on every core.
"""

import numpy as np

import concourse.bass as bass
import concourse.mybir as mybir
import concourse.tile as tile
from concourse import bacc, bass_utils

F32 = mybir.dt.float32
BF16 = mybir.dt.bfloat16
NSQ = 4                    # ReduceScatter split: one piece per 2 s-chunks
AX = mybir.AxisListType
ALU = mybir.AluOpType
ACTF = mybir.ActivationFunctionType

B, S, D = 2, 4096, 2048
N_CORES = 8
NG, GS = 2, 4              # batch groups x spectrum shards
FP = 1024                  # packed rfft bins (col0: re=DC, im=Nyquist)
FBLK = FP // GS            # 256 packed bins per core
NFT = FBLK // 128          # 2 local f-tiles
ND = D // 128              # 16 d tiles
NE = D // 128              # 16 e tiles
DQ = D // GS               # 512 output d rows per core
SCH = 512                  # sequence chunk for the pipeline
NSC = S // SCH             # 8 chunks
EPS = 1e-8
# mat order: (name, use_sin(ci), w_idx, bias_col)
MATS = [("kre", 0, 1, 2), ("kim", 1, 1, 3),
        ("vre", 0, 2, 4), ("vim", 1, 2, 5),
        ("qre", 0, 0, 0), ("qim", 1, 0, 1)]

_CACHE = {}


def _build():
    nc = bacc.Bacc("TRN2", target_bir_lowering=False, debug=False,
                   enable_asserts=False, num_devices=N_CORES)

    hT_in = nc.dram_tensor("ht", [D, S], BF16, kind="ExternalInput").ap()
    # W pre-tiled host-side into the exact SBUF image per d-block:
    # w[dt, p, t*128+c] = W[t*128+p, dt*128+c] -> contiguous 4KB rows
    w_ins = [nc.dram_tensor(f"w{x}", [ND * 128, NE * 128], BF16,
                            kind="ExternalInput").ap() for x in "qkv"]
    csl_in = nc.dram_tensor("csl", [D, 2 * FBLK], BF16, kind="ExternalInput").ap()
    am_in = nc.dram_tensor("am2", [FBLK, D], BF16, kind="ExternalInput").ap()
    bm_in = nc.dram_tensor("bm2", [FBLK, D], BF16, kind="ExternalInput").ap()
    bfc_in = nc.dram_tensor("bfc", [128, NFT * 6], F32, kind="ExternalInput").ap()
    # cols: mz_ft0, mn_ft0, mz_ft1, mn_ft1, gate
    mz_in = nc.dram_tensor("mzg", [128, 5], F32, kind="ExternalInput").ap()
    baseT_in = nc.dram_tensor("baseT", [DQ, S], F32, kind="ExternalInput").ap()
    outT = nc.dram_tensor("outT", [DQ, S], F32, kind="ExternalOutput").ap()

    with nc.allow_low_precision("bf16 spectra; scan state stays fp32"), \
         tile.TileContext(nc) as tc, \
         tc.tile_pool(name="pc", bufs=1) as PC, \
         tc.tile_pool(name="dram", bufs=1, space="DRAM") as DR:

        # ---------- constants ----------
        mz_sb = PC.tile([128, 5], F32, tag="mz")
        nc.sync.dma_start(mz_sb[:], mz_in[:])
        bfc_sb = PC.tile([128, NFT * 6], F32, tag="bfc")
        nc.sync.dma_start(bfc_sb[:], bfc_in[:])
        eps_sb = PC.tile([128, 1], F32, tag="eps")
        nc.vector.memset(eps_sb[:], EPS * EPS)
        zeros_bf = PC.tile([128, SCH], BF16, tag="zer")
        nc.vector.memset(zeros_bf[:], 0.0)

        # ---------- DRAM intermediates ----------
        SQ = S // NSQ
        parts = [DR.tile([D, SQ], BF16, tag=f"pa{q}", name=f"pa{q}")
                 for q in range(NSQ)]
        rss = [DR.tile([DQ, SQ], BF16, tag=f"rs{q}", name=f"rs{q}")
               for q in range(NSQ)]

        # persistent unbound-spectra planes (filled chunk-wise) + iDFT mats
        with tc.tile_pool(name="pgl", bufs=1) as PGL:
            ofp = {}
            for ftl in range(NFT):
                for pi in range(2):
                    ofp[(ftl, pi)] = PGL.tile([128, S], BF16,
                                              tag=f"of{ftl}{pi}",
                                              name=f"of{ftl}{pi}")
            ab = {}
            for ftl in range(NFT):
                amt = PGL.tile([128, D], BF16, tag=f"am{ftl}", name=f"am{ftl}")
                nc.sync.dma_start(amt[:], am_in[ftl * 128:(ftl + 1) * 128, :])
                bmt = PGL.tile([128, D], BF16, tag=f"bm{ftl}", name=f"bm{ftl}")
                nc.sync.dma_start(bmt[:], bm_in[ftl * 128:(ftl + 1) * 128, :])
                ab[ftl] = (amt, bmt)

            # ============ fold: G[d,f-slice] = W.T @ [C|S] ============
            with tc.tile_pool(name="pgG", bufs=1) as PGG:
                G = [PGG.tile([128, ND * 512], BF16, tag=f"G{wi}",
                              name=f"G{wi}") for wi in range(3)]
                with tc.tile_pool(name="pf", bufs=3) as PF, \
                     tc.tile_pool(name="psf", bufs=1, space="PSUM") as PPF:
                    csl_sb = PF.tile([128, NE * 512], BF16, tag="csl", bufs=1)
                    nc.sync.dma_start(
                        csl_sb[:].rearrange("p (t c) -> p t c", c=512),
                        csl_in.rearrange("(t p) c -> p t c", p=128))
                    for wi in range(3):
                        for dt in range(ND):
                            w_sb = PF.tile([128, NE * 128], BF16, tag="wsl")
                            nc.sync.dma_start(
                                w_sb[:],
                                w_ins[wi][dt * 128:(dt + 1) * 128, :])
                            psf = PPF.tile([128, 512], F32, tag=f"pf{dt % 2}")
                            for e in range(NE):
                                nc.tensor.matmul(
                                    psf[:],
                                    w_sb[:, e * 128:(e + 1) * 128],
                                    csl_sb[:, e * 512:(e + 1) * 512],
                                    start=(e == 0), stop=(e == NE - 1))
                            nc.scalar.copy(
                                G[wi][:, dt * 512:(dt + 1) * 512], psf[:])

                # ============ s-chunk pipeline ============
                with tc.tile_pool(name="pm", bufs=2) as PM, \
                     tc.tile_pool(name="psm", bufs=1, space="PSUM") as PPM:
                    m_prev = {}
                    ofv = {}

                    def load_htc(sc):
                        htc = PM.tile([128, NE * SCH], BF16, tag="htc")
                        nc.sync.dma_start(
                            htc[:].rearrange("p (t s) -> p t s", s=SCH),
                            hT_in[:, sc * SCH:(sc + 1) * SCH]
                            .rearrange("(t p) s -> p t s", p=128))
                        return htc

                    def irfft_sc(sc):
                        # full-depth partial inverse DFT for one s-chunk;
                        # lands in the s-quarter partial tensor sc // 2
                        part = parts[sc // 2]
                        pcol = (sc % 2) * SCH
                        for dt in range(ND):
                            psi = PPM.tile([128, SCH], F32,
                                           tag=f"ir{dt % 2}")
                            step = 0
                            for ftl in range(NFT):
                                amt, bmt = ab[ftl]
                                for pi, abt in ((0, amt), (1, bmt)):
                                    nc.tensor.matmul(
                                        psi[:],
                                        abt[:, dt * 128:(dt + 1) * 128],
                                        ofv[(sc, ftl, pi)],
                                        start=(step == 0),
                                        stop=(step == 2 * NFT - 1))
                                    step += 1
                            stg = PM.tile([128, SCH], BF16, tag=f"sta{dt % 2}")
                            nc.scalar.copy(stg[:], psi[:])
                            nc.scalar.dma_start(
                                part[dt * 128:(dt + 1) * 128,
                                     pcol:pcol + SCH], stg[:])

                    def rs_q(q):
                        nc.gpsimd.collective_compute(
                            "ReduceScatter", ALU.add,
                            replica_groups=[[0, 1, 2, 3], [4, 5, 6, 7]],
                            ins=[parts[q].opt()], outs=[rss[q].opt()])

                    htc = load_htc(0)
                    for sc in range(NSC):
                        s0, s1 = sc * SCH, (sc + 1) * SCH
                        htc_next = load_htc(sc + 1) if sc + 1 < NSC else None
                        # an older chunk's partial iDFT keeps PE busy while
                        # the DVE works through the recent chunks' planes
                        if sc > 1:
                            irfft_sc(sc - 2)
                            if sc % 2 == 1:
                                rs_q(sc // 2 - 1)   # quarter complete
                        planes = {}
                        for mi, (_, ci, wi, bcol) in enumerate(MATS):
                            for ftl in range(NFT):
                                ps = PPM.tile([128, SCH], F32,
                                              tag=f"pp{(2 * mi + ftl) % 2}")
                                off = ci * 256 + ftl * 128
                                for dt in range(ND):
                                    nc.tensor.matmul(
                                        ps[:],
                                        G[wi][:, dt * 512 + off:
                                              dt * 512 + off + 128],
                                        htc[:, dt * SCH:(dt + 1) * SCH],
                                        start=(dt == 0), stop=(dt == ND - 1))
                                pl = PM.tile([128, SCH], BF16,
                                             tag=f"pl{mi}_{ftl}")
                                nc.scalar.activation(
                                    pl[:], ps[:], ACTF.Identity,
                                    bias=bfc_sb[:, ftl * 6 + bcol:
                                                ftl * 6 + bcol + 1])
                                planes[(mi, ftl)] = pl
                        for ftl in range(NFT):
                            mz = mz_sb[:, 2 * ftl:2 * ftl + 1]
                            mn = mz_sb[:, 2 * ftl + 1:2 * ftl + 2]
                            kre, kim = planes[(0, ftl)], planes[(1, ftl)]
                            vre, vim = planes[(2, ftl)], planes[(3, ftl)]
                            qre, qim = planes[(4, ftl)], planes[(5, ftl)]
                            # --- unit-magnitude norms (masked for the two
                            # real bins packed in partition 0 of ft 0) ---
                            rr = {}
                            for pj, (re_, im_) in enumerate(((kre, kim),
                                                            (vre, vim))):
                                sq0 = PM.tile([128, SCH], BF16, tag="sq0")
                                sq1 = PM.tile([128, SCH], BF16, tag="sq1")
                                nc.scalar.square(sq0[:], re_[:])
                                nc.scalar.square(sq1[:], im_[:])
                                ra = PM.tile([128, SCH], BF16, tag=f"ra{pj}")
                                rb = PM.tile([128, SCH], BF16, tag=f"rb{pj}")
                                nc.vector.scalar_tensor_tensor(
                                    ra[:], sq1[:], mz, sq0[:],
                                    ALU.mult, ALU.add)
                                nc.vector.scalar_tensor_tensor(
                                    rb[:], sq0[:], mz, sq1[:],
                                    ALU.mult, ALU.add)
                                nc.scalar.activation(ra[:], ra[:], ACTF.Sqrt,
                                                     bias=eps_sb[:])
                                nc.scalar.activation(rb[:], rb[:], ACTF.Sqrt,
                                                     bias=eps_sb[:])
                                nc.vector.reciprocal(ra[:], ra[:])
                                nc.vector.reciprocal(rb[:], rb[:])
                                rr[pj] = (ra, rb)
                            # --- bind: cre+i*cim = Kn * Vn (masked) ---
                            u0 = PM.tile([128, SCH], BF16, tag="u0")
                            u1 = PM.tile([128, SCH], BF16, tag="u1")
                            t0 = PM.tile([128, SCH], BF16, tag="t0")
                            t1 = PM.tile([128, SCH], BF16, tag="t1")
                            cre = PM.tile([128, SCH], BF16, tag="cre")
                            cim = PM.tile([128, SCH], BF16, tag="cim")
                            nc.vector.tensor_mul(u0[:], kre[:], vre[:])
                            nc.vector.tensor_mul(u1[:], kim[:], vim[:])
                            nc.vector.scalar_tensor_tensor(
                                cre[:], u1[:], mn, u0[:], ALU.mult, ALU.add)
                            nc.vector.tensor_mul(t0[:], kre[:], vim[:])
                            nc.vector.tensor_mul(t1[:], kim[:], vre[:])
                            nc.vector.tensor_add(t0[:], t0[:], t1[:])
                            nc.vector.tensor_sub(t0[:], t0[:], u1[:])
                            nc.vector.scalar_tensor_tensor(
                                cim[:], t0[:], mz, u1[:], ALU.mult, ALU.add)
                            kra, krb = rr[0]
                            vra, vrb = rr[1]
                            nc.vector.tensor_mul(kra[:], kra[:], vra[:])
                            nc.vector.tensor_mul(krb[:], krb[:], vrb[:])
                            nc.vector.tensor_mul(cre[:], cre[:], kra[:])
                            nc.vector.tensor_mul(cim[:], cim[:], krb[:])
                            # --- causal scan (fp32 state, bf16 carry) ---
                            ms = []
                            for pi, cv in enumerate((cre, cim)):
                                mt = PM.tile([128, SCH], BF16,
                                             tag=f"m{ftl}{pi}")
                                init = (0.0 if sc == 0
                                        else m_prev[(ftl, pi)][:, SCH - 1:SCH])
                                nc.vector.tensor_tensor_scan(
                                    mt[:], cv[:], zeros_bf[:], init,
                                    ALU.add, ALU.add)
                                m_prev[(ftl, pi)] = mt
                                ms.append(mt)
                            # --- unbind: Of = conj(Qf) * Mf (masked) ---
                            ofv[(sc, ftl, 0)] = ofp[(ftl, 0)][:, s0:s1]
                            ofv[(sc, ftl, 1)] = ofp[(ftl, 1)][:, s0:s1]
                            nc.vector.tensor_mul(u0[:], qre[:], ms[0][:])
                            nc.vector.tensor_mul(u1[:], qim[:], ms[1][:])
                            nc.vector.scalar_tensor_tensor(
                                ofv[(sc, ftl, 0)], u1[:], mz, u0[:],
                                ALU.mult, ALU.add)
                            nc.vector.tensor_mul(t0[:], qre[:], ms[1][:])
                            nc.vector.tensor_mul(t1[:], qim[:], ms[0][:])
                            nc.vector.tensor_sub(t0[:], t0[:], t1[:])
                            nc.vector.tensor_sub(t0[:], t0[:], u1[:])
                            nc.vector.scalar_tensor_tensor(
                                ofv[(sc, ftl, 1)], t0[:], mz, u1[:],
                                ALU.mult, ALU.add)
                        htc = htc_next
                    irfft_sc(NSC - 2)
                    irfft_sc(NSC - 1)
                    rs_q(NSQ - 1)

            # ============ epilogue: out = base + gate * adapter ============
            # s-quarter order: rs outputs arrive q0..q3; base prefetched
            with tc.tile_pool(name="pe", bufs=1) as PE2:
                SQ = S // NSQ
                bss = []
                for t in range(DQ // 128):
                    bs = PE2.tile([128, S], F32, tag=f"bs{t}")
                    nc.sync.dma_start(
                        bs[:], baseT_in[t * 128:(t + 1) * 128, :])
                    bss.append(bs)
                for q in range(NSQ):
                    for t in range(DQ // 128):
                        ad = PE2.tile([128, SQ], BF16, tag=f"ad{t % 2}",
                                      bufs=2)
                        nc.sync.dma_start(
                            ad[:], rss[q][t * 128:(t + 1) * 128, :])
                        ot = PE2.tile([128, SQ], F32, tag=f"ot{t % 2}",
                                      bufs=2)
                        nc.vector.scalar_tensor_tensor(
                            ot[:], ad[:], mz_sb[:, 4:5],
                            bss[t][:, q * SQ:(q + 1) * SQ],
                            ALU.mult, ALU.add)
                        nc.scalar.dma_start(
                            outT[t * 128:(t + 1) * 128, q * SQ:(q + 1) * SQ],
                            ot[:])

    nc.compile()
    return nc


def _constants():
    npbf = mybir.dt.np(BF16)
    e = np.arange(D, dtype=np.float64)
    f = np.arange(FP, dtype=np.float64)
    ang = 2.0 * np.pi * np.outer(e, f) / D           # [e, f]
    cp = np.cos(ang)
    sp = -np.sin(ang)
    sp[:, 0] = np.cos(np.pi * e)                     # Nyquist packed in im col 0
    w = np.full(FP, 2.0)
    w[0] = 1.0
    angA = 2.0 * np.pi * np.outer(f, e) / D          # [f, d]
    am = (w[:, None] / D) * np.cos(angA)
    bm = -(w[:, None] / D) * np.sin(angA)
    bm[0, :] = np.cos(np.pi * e) / D                 # Nyquist inverse row
    return (cp.astype(npbf), sp.astype(npbf),
            am.astype(npbf), bm.astype(npbf))


def _run(inputs, trace=False):
    if "nc" not in _CACHE:
        _CACHE["nc"] = _build()
    nc = _CACHE["nc"]
    npbf = mybir.dt.np(BF16)
    cp, sp, am, bm = _CACHE.setdefault("const", _constants())

    h = np.asarray(inputs["hidden_states"], np.float32).reshape(B, S, D)
    base = np.asarray(inputs["base_output"], np.float32).reshape(B, S, D)
    gate = float(np.asarray(inputs["gate"], np.float32).reshape(-1)[0])

    bf = np.zeros((FP, 6), np.float32)
    for j, bn in enumerate(("bq", "bk", "bv")):
        spec = np.fft.rfft(np.asarray(inputs[bn], np.float64))
        bf[:FP, 2 * j] = spec.real[:FP].astype(np.float32)
        bf[:FP, 2 * j + 1] = spec.imag[:FP].astype(np.float32)
        bf[0, 2 * j + 1] = np.float32(spec.real[FP])

    # W pre-tiled into the SBUF image: w[dt, p, t*128+c] = W[t*128+p, dt*128+c]
    wt = {}
    for x in "qkv":
        wf = np.asarray(inputs[f"W{x}"], np.float32)
        wt[x] = np.ascontiguousarray(
            wf.reshape(NE, 128, ND, 128).transpose(2, 1, 0, 3)
            .reshape(ND * 128, NE * 128)).astype(npbf)

    hT = [np.ascontiguousarray(h[g].T).astype(npbf) for g in range(NG)]

    in_maps = []
    for c in range(N_CORES):
        g, r = c // GS, c % GS
        blk = slice(r * FBLK, (r + 1) * FBLK)
        csl = np.concatenate([cp[:, blk], sp[:, blk]], axis=1)
        bfc = np.empty((128, NFT * 6), np.float32)
        for ftl in range(NFT):
            bfc[:, ftl * 6:(ftl + 1) * 6] = \
                bf[r * FBLK + ftl * 128: r * FBLK + (ftl + 1) * 128]
        mzg = np.ones((128, 5), np.float32)
        if r == 0:
            mzg[0, 0] = 0.0          # ft0 partition 0: DC/Nyquist real bins
        mzg[:, 1] = -mzg[:, 0]
        mzg[:, 3] = -mzg[:, 2]
        mzg[:, 4] = gate
        baseT = np.ascontiguousarray(base[g][:, DQ * r:DQ * (r + 1)].T)
        in_maps.append({
            "ht": hT[g],
            "wq": wt["q"], "wk": wt["k"], "wv": wt["v"],
            "csl": np.ascontiguousarray(csl),
            "am2": np.ascontiguousarray(am[blk]),
            "bm2": np.ascontiguousarray(bm[blk]),
            "bfc": bfc, "mzg": mzg, "baseT": baseT,
        })

    res = bass_utils.run_bass_kernel_spmd(
        nc, in_maps, core_ids=list(range(N_CORES)), trace=trace)

    out = np.empty((B, S, D), np.float32)
    for c in range(N_CORES):
        g, r = c // GS, c % GS
        out[g][:, DQ * r:DQ * (r + 1)] = res.results[c]["outT"].T
    return out, res


def kernel(**inputs) -> np.ndarray:
    out, _ = _run(inputs)
    return out


# revision 32
# speedup vs baseline: 3.2434x; 1.0256x over previous
"""HRR adapted attention kernel for 8 trn2 cores.

Math (verified vs reference in f64):
  q,k,v = h @ W{q,k,v}.T + b      (per-row, D=2048)
  Qf = rfft(q); Kf = rfft(k)/(|rfft(k)|+eps); Vf likewise
  Mf = causal-cumsum_S(Kf*Vf);  Of = conj(Qf)*Mf;  adapter = irfft(Of)
  out = base + gate*adapter

All FFTs become matmuls: the DFT folds into the projections,
G = W.T @ [C|S] in [d,f] orientation, so the Q/K/V spectra come straight
out of hT.T @ G in a freq-on-partition layout where the causal cumsum is
a native tensor_tensor_scan along the free (sequence) dim.

Sharding: 2 batch groups x 4 spectrum shards. Core c handles batch c//4
and 256 packed rfft bins (2 f-tiles of 128). Each core folds only its own
spectrum slice (full W needed, no fold collective), projects / binds /
scans all 4096 rows of its batch locally, and computes a PARTIAL inverse
DFT over its f-slice. One grouped bf16 ReduceScatter (d-sharded, split in
two halves for overlap) combines the partials; the epilogue adds base in
the transposed [d, s] layout so no on-chip transposes are needed at all.

The packed spectrum keeps rfft bins DC and Nyquist in the re/im planes of
packed column 0 (both real). Their special normalize/bind/unbind algebra
is expressed uniformly via per-partition {0,1} mask columns, so the SPMD
program is identical on every core.
"""

import numpy as np

import concourse.bass as bass
import concourse.mybir as mybir
import concourse.tile as tile
from concourse import bacc, bass_utils

F32 = mybir.dt.float32
BF16 = mybir.dt.bfloat16
NSQ = 4                    # ReduceScatter split: one piece per 2 s-chunks
AX = mybir.AxisListType
ALU = mybir.AluOpType
ACTF = mybir.ActivationFunctionType

B, S, D = 2, 4096, 2048
N_CORES = 8
NG, GS = 2, 4              # batch groups x spectrum shards
FP = 1024                  # packed rfft bins (col0: re=DC, im=Nyquist)
FBLK = FP // GS            # 256 packed bins per core
NFT = FBLK // 128          # 2 local f-tiles
ND = D // 128              # 16 d tiles
NE = D // 128              # 16 e tiles
DQ = D // GS               # 512 output d rows per core
SCH = 512                  # sequence chunk for the pipeline
NSC = S // SCH             # 8 chunks
EPS = 1e-8
# mat order: (name, use_sin(ci), w_idx, bias_col)
MATS = [("kre", 0, 1, 2), ("kim", 1, 1, 3),
        ("vre", 0, 2, 4), ("vim", 1, 2, 5),
        ("qre", 0, 0, 0), ("qim", 1, 0, 1)]

_CACHE = {}


def _build():
    nc = bacc.Bacc("TRN2", target_bir_lowering=False, debug=False,
                   enable_asserts=False, num_devices=N_CORES)

    hT_in = nc.dram_tensor("ht", [D, S], BF16, kind="ExternalInput").ap()
    # W pre-tiled host-side into the exact SBUF image per d-block:
    # w[dt, p, t*128+c] = W[t*128+p, dt*128+c] -> contiguous 4KB rows
    w_ins = [nc.dram_tensor(f"w{x}", [ND * 128, NE * 128], BF16,
                            kind="ExternalInput").ap() for x in "qkv"]
    csl_in = nc.dram_tensor("csl", [D, 2 * FBLK], BF16, kind="ExternalInput").ap()
    am_in = nc.dram_tensor("am2", [FBLK, D], BF16, kind="ExternalInput").ap()
    bm_in = nc.dram_tensor("bm2", [FBLK, D], BF16, kind="ExternalInput").ap()
    bfc_in = nc.dram_tensor("bfc", [128, NFT * 6], F32, kind="ExternalInput").ap()
    # cols: mz_ft0, mn_ft0, mz_ft1, mn_ft1, gate
    mz_in = nc.dram_tensor("mzg", [128, 5], F32, kind="ExternalInput").ap()
    baseT_in = nc.dram_tensor("baseT", [DQ, S], F32, kind="ExternalInput").ap()
    outT = nc.dram_tensor("outT", [DQ, S], F32, kind="ExternalOutput").ap()

    with nc.allow_low_precision("bf16 spectra; scan state stays fp32"), \
         tile.TileContext(nc) as tc, \
         tc.tile_pool(name="pc", bufs=1) as PC, \
         tc.tile_pool(name="dram", bufs=1, space="DRAM") as DR:

        # ---------- constants ----------
        mz_sb = PC.tile([128, 5], F32, tag="mz")
        nc.sync.dma_start(mz_sb[:], mz_in[:])
        bfc_sb = PC.tile([128, NFT * 6], F32, tag="bfc")
        nc.sync.dma_start(bfc_sb[:], bfc_in[:])
        eps_sb = PC.tile([128, 1], F32, tag="eps")
        nc.vector.memset(eps_sb[:], EPS * EPS)
        zeros_bf = PC.tile([128, SCH], BF16, tag="zer")
        nc.vector.memset(zeros_bf[:], 0.0)

        # ---------- DRAM intermediates ----------
        SQ = S // NSQ
        parts = [DR.tile([D, SQ], BF16, tag=f"pa{q}", name=f"pa{q}")
                 for q in range(NSQ)]
        rss = [DR.tile([DQ, SQ], BF16, tag=f"rs{q}", name=f"rs{q}")
               for q in range(NSQ)]

        # persistent unbound-spectra planes (filled chunk-wise) + iDFT mats
        with tc.tile_pool(name="pgl", bufs=1) as PGL:
            ofp = {}
            for ftl in range(NFT):
                for pi in range(2):
                    ofp[(ftl, pi)] = PGL.tile([128, S], BF16,
                                              tag=f"of{ftl}{pi}",
                                              name=f"of{ftl}{pi}")
            ab = {}

            # ============ fold: G[d,f-slice] = W.T @ [C|S] ============
            with tc.tile_pool(name="pgG", bufs=1) as PGG, \
                 tc.tile_pool(name="pht", bufs=2) as PHT:

                def load_htc(sc):
                    htc = PHT.tile([128, NE * SCH], BF16, tag="htc")
                    nc.sync.dma_start(
                        htc[:].rearrange("p (t s) -> p t s", s=SCH),
                        hT_in[:, sc * SCH:(sc + 1) * SCH]
                        .rearrange("(t p) s -> p t s", p=128))
                    return htc

                G = [PGG.tile([128, ND * 512], BF16, tag=f"G{wi}",
                              name=f"G{wi}") for wi in range(3)]
                with tc.tile_pool(name="pf", bufs=3) as PF, \
                     tc.tile_pool(name="psf", bufs=1, space="PSUM") as PPF:
                    csl_sb = PF.tile([128, NE * 512], BF16, tag="csl", bufs=1)
                    nc.sync.dma_start(
                        csl_sb[:].rearrange("p (t c) -> p t c", c=512),
                        csl_in.rearrange("(t p) c -> p t c", p=128))
                    htc0 = load_htc(0)
                    for wi in range(3):
                        for dt in range(ND):
                            w_sb = PF.tile([128, NE * 128], BF16, tag="wsl")
                            nc.sync.dma_start(
                                w_sb[:],
                                w_ins[wi][dt * 128:(dt + 1) * 128, :])
                            psf = PPF.tile([128, 512], F32, tag=f"pf{dt % 2}")
                            for e in range(NE):
                                nc.tensor.matmul(
                                    psf[:],
                                    w_sb[:, e * 128:(e + 1) * 128],
                                    csl_sb[:, e * 512:(e + 1) * 512],
                                    start=(e == 0), stop=(e == NE - 1))
                            nc.scalar.copy(
                                G[wi][:, dt * 512:(dt + 1) * 512], psf[:])
                    for ftl in range(NFT):
                        amt = PGL.tile([128, D], BF16, tag=f"am{ftl}",
                                       name=f"am{ftl}")
                        nc.sync.dma_start(
                            amt[:], am_in[ftl * 128:(ftl + 1) * 128, :])
                        bmt = PGL.tile([128, D], BF16, tag=f"bm{ftl}",
                                       name=f"bm{ftl}")
                        nc.sync.dma_start(
                            bmt[:], bm_in[ftl * 128:(ftl + 1) * 128, :])
                        ab[ftl] = (amt, bmt)

                # ============ s-chunk pipeline ============
                with tc.tile_pool(name="pm", bufs=2) as PM, \
                     tc.tile_pool(name="psm", bufs=1, space="PSUM") as PPM:
                    m_prev = {}
                    ofv = {}

                    def irfft_sc(sc):
                        # full-depth partial inverse DFT for one s-chunk;
                        # lands in the s-quarter partial tensor sc // 2
                        part = parts[sc // 2]
                        pcol = (sc % 2) * SCH
                        for dt in range(ND):
                            psi = PPM.tile([128, SCH], F32,
                                           tag=f"ir{dt % 4}")
                            step = 0
                            for ftl in range(NFT):
                                amt, bmt = ab[ftl]
                                for pi, abt in ((0, amt), (1, bmt)):
                                    nc.tensor.matmul(
                                        psi[:],
                                        abt[:, dt * 128:(dt + 1) * 128],
                                        ofv[(sc, ftl, pi)],
                                        start=(step == 0),
                                        stop=(step == 2 * NFT - 1))
                                    step += 1
                            stg = PM.tile([128, SCH], BF16, tag=f"sta{dt % 2}")
                            nc.scalar.copy(stg[:], psi[:])
                            nc.scalar.dma_start(
                                part[dt * 128:(dt + 1) * 128,
                                     pcol:pcol + SCH], stg[:])

                    def rs_q(q):
                        nc.gpsimd.collective_compute(
                            "ReduceScatter", ALU.add,
                            replica_groups=[[0, 1, 2, 3], [4, 5, 6, 7]],
                            ins=[parts[q].opt()], outs=[rss[q].opt()])

                    htc = htc0
                    for sc in range(NSC):
                        s0, s1 = sc * SCH, (sc + 1) * SCH
                        htc_next = load_htc(sc + 1) if sc + 1 < NSC else None
                        # an older chunk's partial iDFT keeps PE busy while
                        # the DVE works through the recent chunks' planes
                        if sc > 1:
                            irfft_sc(sc - 2)
                            if sc % 2 == 1:
                                rs_q(sc // 2 - 1)   # quarter complete
                        planes = {}
                        for mi, (_, ci, wi, bcol) in enumerate(MATS):
                            for ftl in range(NFT):
                                ps = PPM.tile([128, SCH], F32,
                                              tag=f"pp{(2 * mi + ftl) % 3}")
                                off = ci * 256 + ftl * 128
                                for dt in range(ND):
                                    nc.tensor.matmul(
                                        ps[:],
                                        G[wi][:, dt * 512 + off:
                                              dt * 512 + off + 128],
                                        htc[:, dt * SCH:(dt + 1) * SCH],
                                        start=(dt == 0), stop=(dt == ND - 1))
                                pl = PM.tile([128, SCH], BF16,
                                             tag=f"pl{mi}_{ftl}")
                                nc.scalar.activation(
                                    pl[:], ps[:], ACTF.Identity,
                                    bias=bfc_sb[:, ftl * 6 + bcol:
                                                ftl * 6 + bcol + 1])
                                planes[(mi, ftl)] = pl
                        for ftl in range(NFT):
                            mz = mz_sb[:, 2 * ftl:2 * ftl + 1]
                            mn = mz_sb[:, 2 * ftl + 1:2 * ftl + 2]
                            kre, kim = planes[(0, ftl)], planes[(1, ftl)]
                            vre, vim = planes[(2, ftl)], planes[(3, ftl)]
                            qre, qim = planes[(4, ftl)], planes[(5, ftl)]
                            # --- unit-magnitude norms (masked for the two
                            # real bins packed in partition 0 of ft 0) ---
                            rr = {}
                            for pj, (re_, im_) in enumerate(((kre, kim),
                                                            (vre, vim))):
                                sq0 = PM.tile([128, SCH], BF16, tag="sq0")
                                sq1 = PM.tile([128, SCH], BF16, tag="sq1")
                                nc.scalar.square(sq0[:], re_[:])
                                nc.scalar.square(sq1[:], im_[:])
                                ra = PM.tile([128, SCH], BF16, tag=f"ra{pj}")
                                rb = PM.tile([128, SCH], BF16, tag=f"rb{pj}")
                                nc.vector.scalar_tensor_tensor(
                                    ra[:], sq1[:], mz, sq0[:],
                                    ALU.mult, ALU.add)
                                nc.vector.scalar_tensor_tensor(
                                    rb[:], sq0[:], mz, sq1[:],
                                    ALU.mult, ALU.add)
                                nc.scalar.activation(ra[:], ra[:], ACTF.Sqrt,
                                                     bias=eps_sb[:])
                                nc.scalar.activation(rb[:], rb[:], ACTF.Sqrt,
                                                     bias=eps_sb[:])
                                nc.vector.reciprocal(ra[:], ra[:])
                                nc.vector.reciprocal(rb[:], rb[:])
                                rr[pj] = (ra, rb)
                            # --- bind: cre+i*cim = Kn * Vn (masked) ---
                            u0 = PM.tile([128, SCH], BF16, tag="u0")
                            u1 = PM.tile([128, SCH], BF16, tag="u1")
                            t0 = PM.tile([128, SCH], BF16, tag="t0")
                            t1 = PM.tile([128, SCH], BF16, tag="t1")
                            cre = PM.tile([128, SCH], BF16, tag="cre")
                            cim = PM.tile([128, SCH], BF16, tag="cim")
                            nc.vector.tensor_mul(u0[:], kre[:], vre[:])
                            nc.vector.tensor_mul(u1[:], kim[:], vim[:])
                            nc.vector.scalar_tensor_tensor(
                                cre[:], u1[:], mn, u0[:], ALU.mult, ALU.add)
                            nc.vector.tensor_mul(t0[:], kre[:], vim[:])
                            nc.vector.tensor_mul(t1[:], kim[:], vre[:])
                            nc.vector.tensor_add(t0[:], t0[:], t1[:])
                            nc.vector.tensor_sub(t0[:], t0[:], u1[:])
                            nc.vector.scalar_tensor_tensor(
                                cim[:], t0[:], mz, u1[:], ALU.mult, ALU.add)
                            kra, krb = rr[0]
                            vra, vrb = rr[1]
                            nc.vector.tensor_mul(kra[:], kra[:], vra[:])
                            nc.vector.tensor_mul(krb[:], krb[:], vrb[:])
                            nc.vector.tensor_mul(cre[:], cre[:], kra[:])
                            nc.vector.tensor_mul(cim[:], cim[:], krb[:])
                            # --- causal scan (fp32 state, bf16 carry) ---
                            ms = []
                            for pi, cv in enumerate((cre, cim)):
                                mt = PM.tile([128, SCH], BF16,
                                             tag=f"m{ftl}{pi}")
                                init = (0.0 if sc == 0
                                        else m_prev[(ftl, pi)][:, SCH - 1:SCH])
                                nc.vector.tensor_tensor_scan(
                                    mt[:], cv[:], zeros_bf[:], init,
                                    ALU.add, ALU.add)
                                m_prev[(ftl, pi)] = mt
                                ms.append(mt)
                            # --- unbind: Of = conj(Qf) * Mf (masked) ---
                            ofv[(sc, ftl, 0)] = ofp[(ftl, 0)][:, s0:s1]
                            ofv[(sc, ftl, 1)] = ofp[(ftl, 1)][:, s0:s1]
                            nc.vector.tensor_mul(u0[:], qre[:], ms[0][:])
                            nc.vector.tensor_mul(u1[:], qim[:], ms[1][:])
                            nc.vector.scalar_tensor_tensor(
                                ofv[(sc, ftl, 0)], u1[:], mz, u0[:],
                                ALU.mult, ALU.add)
                            nc.vector.tensor_mul(t0[:], qre[:], ms[1][:])
                            nc.vector.tensor_mul(t1[:], qim[:], ms[0][:])
                            nc.vector.tensor_sub(t0[:], t0[:], t1[:])
                            nc.vector.tensor_sub(t0[:], t0[:], u1[:])
                            nc.vector.scalar_tensor_tensor(
                                ofv[(sc, ftl, 1)], t0[:], mz, u1[:],
                                ALU.mult, ALU.add)
                        htc = htc_next
                    irfft_sc(NSC - 2)
                    irfft_sc(NSC - 1)
                    rs_q(NSQ - 1)

            # ============ epilogue: out = base + gate * adapter ============
            # per-(quarter, d-tile) pieces; all dep-free base loads are
            # issued up front so only the rs reads sit behind the collectives
            with tc.tile_pool(name="pe", bufs=1) as PE2:
                SQ = S // NSQ
                bss = {}
                for q in range(NSQ):
                    for t in range(DQ // 128):
                        bs = PE2.tile([128, SQ], F32, tag=f"bs{q}_{t}")
                        nc.sync.dma_start(
                            bs[:], baseT_in[t * 128:(t + 1) * 128,
                                            q * SQ:(q + 1) * SQ])
                        bss[(q, t)] = bs
                for q in range(NSQ):
                    for t in range(DQ // 128):
                        ad = PE2.tile([128, SQ], BF16, tag=f"ad{t % 2}",
                                      bufs=2)
                        nc.sync.dma_start(
                            ad[:], rss[q][t * 128:(t + 1) * 128, :])
                        ot = PE2.tile([128, SQ], F32, tag=f"ot{t % 2}",
                                      bufs=2)
                        nc.vector.scalar_tensor_tensor(
                            ot[:], ad[:], mz_sb[:, 4:5], bss[(q, t)][:],
                            ALU.mult, ALU.add)
                        nc.scalar.dma_start(
                            outT[t * 128:(t + 1) * 128, q * SQ:(q + 1) * SQ],
                            ot[:])

    nc.compile()
    return nc


def _constants():
    npbf = mybir.dt.np(BF16)
    e = np.arange(D, dtype=np.float64)
    f = np.arange(FP, dtype=np.float64)
    ang = 2.0 * np.pi * np.outer(e, f) / D           # [e, f]
    cp = np.cos(ang)
    sp = -np.sin(ang)
    sp[:, 0] = np.cos(np.pi * e)                     # Nyquist packed in im col 0
    w = np.full(FP, 2.0)
    w[0] = 1.0
    angA = 2.0 * np.pi * np.outer(f, e) / D          # [f, d]
    am = (w[:, None] / D) * np.cos(angA)
    bm = -(w[:, None] / D) * np.sin(angA)
    bm[0, :] = np.cos(np.pi * e) / D                 # Nyquist inverse row
    return (cp.astype(npbf), sp.astype(npbf),
            am.astype(npbf), bm.astype(npbf))


def _run(inputs, trace=False):
    if "nc" not in _CACHE:
        _CACHE["nc"] = _build()
    nc = _CACHE["nc"]
    npbf = mybir.dt.np(BF16)
    cp, sp, am, bm = _CACHE.setdefault("const", _constants())

    h = np.asarray(inputs["hidden_states"], np.float32).reshape(B, S, D)
    base = np.asarray(inputs["base_output"], np.float32).reshape(B, S, D)
    gate = float(np.asarray(inputs["gate"], np.float32).reshape(-1)[0])

    bf = np.zeros((FP, 6), np.float32)
    for j, bn in enumerate(("bq", "bk", "bv")):
        spec = np.fft.rfft(np.asarray(inputs[bn], np.float64))
        bf[:FP, 2 * j] = spec.real[:FP].astype(np.float32)
        bf[:FP, 2 * j + 1] = spec.imag[:FP].astype(np.float32)
        bf[0, 2 * j + 1] = np.float32(spec.real[FP])

    # W pre-tiled into the SBUF image: w[dt, p, t*128+c] = W[t*128+p, dt*128+c]
    wt = {}
    for x in "qkv":
        wf = np.asarray(inputs[f"W{x}"], np.float32)
        wt[x] = np.ascontiguousarray(
            wf.reshape(NE, 128, ND, 128).transpose(2, 1, 0, 3)
            .reshape(ND * 128, NE * 128)).astype(npbf)

    hT = [np.ascontiguousarray(h[g].T).astype(npbf) for g in range(NG)]

    in_maps = []
    for c in range(N_CORES):
        g, r = c // GS, c % GS
        blk = slice(r * FBLK, (r + 1) * FBLK)
        csl = np.concatenate([cp[:, blk], sp[:, blk]], axis=1)
        bfc = np.empty((128, NFT * 6), np.float32)
        for ftl in range(NFT):
            bfc[:, ftl * 6:(ftl + 1) * 6] = \
                bf[r * FBLK + ftl * 128: r * FBLK + (ftl + 1) * 128]
        mzg = np.ones((128, 5), np.float32)
        if r == 0:
            mzg[0, 0] = 0.0          # ft0 partition 0: DC/Nyquist real bins
        mzg[:, 1] = -mzg[:, 0]
        mzg[:, 3] = -mzg[:, 2]
        mzg[:, 4] = gate
        baseT = np.ascontiguousarray(base[g][:, DQ * r:DQ * (r + 1)].T)
        in_maps.append({
            "ht": hT[g],
            "wq": wt["q"], "wk": wt["k"], "wv": wt["v"],
            "csl": np.ascontiguousarray(csl),
            "am2": np.ascontiguousarray(am[blk]),
            "bm2": np.ascontiguousarray(bm[blk]),
            "bfc": bfc, "mzg": mzg, "baseT": baseT,
        })

    res = bass_utils.run_bass_kernel_spmd(
        nc, in_maps, core_ids=list(range(N_CORES)), trace=trace)

    out = np.empty((B, S, D), np.float32)
    for c in range(N_CORES):
        g, r = c // GS, c % GS
        out[g][:, DQ * r:DQ * (r + 1)] = res.results[c]["outT"].T
    return out, res


def kernel(**inputs) -> np.ndarray:
    out, _ = _run(inputs)
    return out
